# revision 1
# baseline (speedup 1.0000x reference)
"""Laplace attention kernel for Trainium2, 8 NeuronCores.

Math (per batch b):
  k = MLP_k(x1[b])  [NK, D];  q = MLP_q(x2[b])  [NQ, D]
  dist[i,j] = sum_d |k[j,d] - q[i,d]|
  out = softmax_j(-dist) @ r[b]

Distribution: core c = (b, h) = (c//2, c%2): batch b, query-half h (256 queries).
Keys/values are replicated per batch pair of cores.

Per-core algorithm:
  - MLPs run transposed on the PE: kT2 [128=(i2,d), NK] holds kT stacked twice,
    q2T [128=(i2,d), 128] holds qT for queries (i2*128 + p).
  - For each query pair p, a [128, NK] tile M_p is produced:
      min-form pairs (DVE):  M_p = min(kT2, q_p)        (tensor_scalar, 2x fp32)
      abs-form pairs (ACT):  M_p = |kT2 - q_p|          (activation Abs, bias=q, scale=-1)
    Then one PE matmul per 512-column window reduces over the 128 partitions
    with a constant ones-block lhsT whose coefficient is -2 for min-form
    columns and +1 for abs-form columns, accumulating 32 pairs into one
    [64, 1024] PSUM tile. A K=1 correction matmul adds A_j = sum_d k[j,d]
    to min-form rows only (masked lhsT). The remaining B_i = sum_d q[i,d]
    offset is constant per row and cancels in softmax, and dist is large
    and positive, so exp(-P) needs no max-subtraction.
  - softmax: ACT Exp (scale=-1) with accum_out row-sum, DVE reciprocal +
    scale; weights stored bf16.
  - value matmul: DMA-transpose the bf16 weights to [j, q] layout, then PE
    per 128-key tile with r (bf16) as stationary operand, accumulating in
    PSUM; result is [D, queries], written out and transposed on the host.
"""

import os
import numpy as np
import ml_dtypes

import concourse.bass as bass
import concourse.mybir as mybir
from concourse.tile import TileContext
from concourse import bass_utils

B, NQ, NK, D = 4, 512, 1024, 64
NCORES = 8
QSH = NQ // 2           # queries per core
NPAIR = QSH // 2        # 128 query pairs per core
NWIN = NK // 512        # 512-column matmul windows
ACT_SLOTS = (1, 4, 8, 11, 15, 18, 22, 25, 29)  # slots within later groups

F32 = mybir.dt.float32
F32R = mybir.dt.float32r
F16 = mybir.dt.float16
BF16 = mybir.dt.bfloat16

LAST_RESULT = None      # BassKernelResults of the most recent run (for test.py)



def _is_act_pair(p):
    # no ACT pairs among the first few: their Mt tiles must fill the pipeline
    # while the ACT engine is still busy with the MLP chain
    return p >= 8 and (p % 32) in ACT_SLOTS


# ---------------------------------------------------------------------------
# walrus workaround: the CTRL-class instructions (Drain etc.) can carry only a
# few sem waits; hoist excess waits onto injected NoOps on the same engine.
def _split_excess_waits(nc, max_waits=1):
    for f in nc.m.functions:
        for bb in f.blocks:
            new_insts = []
            for inst in bb.instructions:
                si = inst.sync_info
                if si is not None and si.on_wait and len(si.on_wait) > max_waits:
                    waits = list(si.on_wait)
                    excess, keep = waits[:-max_waits], waits[-max_waits:]
                    for i in range(0, len(excess), max_waits):
                        nop = mybir.InstNoOp(
                            name=f"{inst.name}_waitsplit_{i // max_waits}",
                            ins=[], outs=[])
                        nop.engine = inst.engine
                        nop.sync_info = mybir.SyncInfo(
                            on_wait=excess[i:i + max_waits], on_update=[])
                        new_insts.append(nop)
                    si.on_wait = keep
                new_insts.append(inst)
            bb.instructions = new_insts


# shim antenv.axon_hooks (absent in this image) so BASS_TRACE=1 profiling works
def _install_ntff_shim():
    import sys, types
    if 'antenv.axon_hooks' in sys.modules:
        return
    try:
        mod = types.ModuleType('antenv.axon_hooks')
        state = {}
        mod.set_axon_ntff_profile_hook = lambda h: state.__setitem__('h', h)
        mod.get_axon_ntff_profile_hook = lambda: state.get('h')
        sys.modules['antenv.axon_hooks'] = mod
        import antenv
        antenv.axon_hooks = mod
        from trn_agent_boot.trn_boot import _ntff_profile_via_ctypes
        h = _ntff_profile_via_ctypes('/opt/axon/libaxon_pjrt.so')
        if h is not None:
            mod.set_axon_ntff_profile_hook(h)
    except Exception:
        pass


# ---------------------------------------------------------------------------
def _build_program():
    nc = bass.Bass("TRN2")

    x1t = nc.dram_tensor("x1t", [D, NK], F16, kind="ExternalInput")
    x2t = nc.dram_tensor("x2t", [D, QSH], F16, kind="ExternalInput")
    rv = nc.dram_tensor("rv", [NK, D], BF16, kind="ExternalInput")
    wk1 = nc.dram_tensor("wk1", [D, D], F16, kind="ExternalInput")
    bk1 = nc.dram_tensor("bk1", [D, 1], F32, kind="ExternalInput")
    wk2d = nc.dram_tensor("wk2d", [D, 128], F16, kind="ExternalInput")
    bk2d = nc.dram_tensor("bk2d", [128, 1], F32, kind="ExternalInput")
    wq1 = nc.dram_tensor("wq1", [D, D], F16, kind="ExternalInput")
    bq1 = nc.dram_tensor("bq1", [D, 1], F32, kind="ExternalInput")
    wq2 = nc.dram_tensor("wq2", [D, D], F16, kind="ExternalInput")
    bq2d = nc.dram_tensor("bq2d", [128, 1], F32, kind="ExternalInput")
    wones = nc.dram_tensor("wones", [128, 64 * 64], F16, kind="ExternalInput")
    cmask = nc.dram_tensor("cmask", [1, 192], F16, kind="ExternalInput")
    ones64 = nc.dram_tensor("ones64", [D, 1], F16, kind="ExternalInput")
    ident = nc.dram_tensor("ident", [128, 128], BF16, kind="ExternalInput")
    yout = nc.dram_tensor("yout", [2, D, 128], F32, kind="ExternalOutput")
    sout = nc.dram_tensor("sout", [2, 128], F32, kind="ExternalOutput")

    ACT = mybir.ActivationFunctionType
    ALU = mybir.AluOpType

    with TileContext(nc) as tc:
        import contextlib
        with contextlib.ExitStack() as ctx:
            consts = ctx.enter_context(tc.tile_pool(name="consts", bufs=1))

            x1t_sb = consts.tile([D, NK], F16)
            x2t_sb = consts.tile([D, QSH], F16)
            r_sb = consts.tile([128, 8 * D], BF16)
            wk1_sb = consts.tile([D, D], F16)
            bk1_sb = consts.tile([D, 1], F32)
            wk2d_sb = consts.tile([D, 128], F16)
            bk2d_sb = consts.tile([128, 1], F32)
            wq1_sb = consts.tile([D, D], F16)
            bq1_sb = consts.tile([D, 1], F32)
            wq2_sb = consts.tile([D, D], F16)
            bq2d_sb = consts.tile([128, 1], F32)
            wones_sb = consts.tile([128, 64 * 64], F16)
            cmask_sb = consts.tile([1, 192], F16)
            ones64_sb = consts.tile([D, 1], F16)
            ident_sb = consts.tile([128, 128], BF16)

            nc.scalar.dma_start(out=wq1_sb[:], in_=wq1[:, :])
            nc.scalar.dma_start(out=bq1_sb[:], in_=bq1[:, :])
            nc.scalar.dma_start(out=x2t_sb[:], in_=x2t[:, :])
            nc.scalar.dma_start(out=wq2_sb[:], in_=wq2[:, :])
            nc.scalar.dma_start(out=bq2d_sb[:], in_=bq2d[:, :])
            nc.sync.dma_start(out=x1t_sb[:, 0:512], in_=x1t[:, 0:512])
            nc.scalar.dma_start(out=x1t_sb[:, 512:1024], in_=x1t[:, 512:1024])
            nc.sync.dma_start(out=wk1_sb[:], in_=wk1[:, :])
            nc.sync.dma_start(out=bk1_sb[:], in_=bk1[:, :])
            nc.sync.dma_start(out=wk2d_sb[:], in_=wk2d[:, :])
            nc.sync.dma_start(out=bk2d_sb[:], in_=bk2d[:, :])
            nc.gpsimd.dma_start(out=wones_sb[:], in_=wones[:, :])
            nc.gpsimd.dma_start(out=cmask_sb[:], in_=cmask[:, :])
            nc.gpsimd.dma_start(out=ones64_sb[:], in_=ones64[:, :])
            nc.gpsimd.dma_start(out=ident_sb[:], in_=ident[:, :])
            for jt in range(8):
                nc.gpsimd.dma_start(out=r_sb[:, jt * D:(jt + 1) * D],
                                    in_=rv[jt * 128:(jt + 1) * 128, :])

            kt2_sb = consts.tile([128, NK], F16)
            q2t_sb = consts.tile([128, 128], F32)
            ht_sb = consts.tile([D, NK], F16)
            hqt_sb = consts.tile([D, QSH], F16)
            arow_sb = consts.tile([1, NK], F16)

            # ---- MLPs (transposed) ----
            # q-path first: q2t gates every Mt producer. All evacuations on
            # ACT so the DVE can start min production immediately after.
            with tc.tile_pool(name="mlppsum", bufs=2, space="PSUM") as mp:
                phq = mp.tile([D, QSH], F32, tag="ph")
                nc.tensor.matmul(phq[:], wq1_sb[:], x2t_sb[:], start=True, stop=True)
                nc.scalar.activation(hqt_sb[:], phq[:], ACT.Relu,
                                     bias=bq1_sb[:, 0:1], scale=1.0)
                pq = mp.tile([128, 128], F32, tag="pk")
                nc.tensor.matmul(pq[0:64, :], wq2_sb[:], hqt_sb[:, 0:128],
                                 start=True, stop=False, skip_group_check=True)
                nc.tensor.matmul(pq[64:128, :], wq2_sb[:], hqt_sb[:, 128:256],
                                 start=True, stop=True, skip_group_check=True)
                nc.scalar.activation(q2t_sb[:], pq[:], ACT.Identity,
                                     bias=bq2d_sb[:, 0:1], scale=1.0)
                for w in range(NWIN):
                    ph = mp.tile([D, 512], F32, tag="ph")
                    nc.tensor.matmul(ph[:], wk1_sb[:], x1t_sb[:, w * 512:(w + 1) * 512],
                                     start=True, stop=True)
                    nc.scalar.activation(ht_sb[:, w * 512:(w + 1) * 512], ph[:],
                                         ACT.Relu, bias=bk1_sb[:, 0:1], scale=1.0)
                    pk = mp.tile([128, 512], F32, tag="pk")
                    nc.tensor.matmul(pk[:], wk2d_sb[:], ht_sb[:, w * 512:(w + 1) * 512],
                                     start=True, stop=True)
                    nc.scalar.activation(kt2_sb[:, w * 512:(w + 1) * 512], pk[:],
                                         ACT.Identity, bias=bk2d_sb[:, 0:1], scale=1.0)
                # A_j = sum_d k[j, d] (same fp16 k the min path sees)
                pa = mp.tile([1, NK], F32, tag="pa")
                for w in range(NWIN):
                    nc.tensor.matmul(pa[:, w * 512:(w + 1) * 512], ones64_sb[:],
                                     kt2_sb[0:64, w * 512:(w + 1) * 512],
                                     start=True, stop=True, skip_group_check=True)
                nc.scalar.copy(arow_sb[:], pa[:])

            # ---- main loop ----
            mpool = ctx.enter_context(tc.tile_pool(name="mtiles", bufs=6))
            dpool = ctx.enter_context(
                tc.tile_pool(name="dist", bufs=3, space="PSUM"))
            opool = ctx.enter_context(
                tc.tile_pool(name="outp", bufs=2, space="PSUM"))
            spool = ctx.enter_context(tc.tile_pool(name="smax", bufs=3))
            otpool = ctx.enter_context(tc.tile_pool(name="outs", bufs=2))

            ot_sbs = {}

            def make_tail(rr, dists, final=False):
                state = {}

                def exp0():
                    expw = spool.tile([128, NK], BF16, tag="expw")
                    ssum = spool.tile([128, 1], F32, tag="ssum")
                    state["expw"], state["ssum"] = expw, ssum
                    nc.scalar.activation(expw[0:64, :], dists[0][:], ACT.Exp,
                                         bias=0.0, scale=-1.0,
                                         accum_out=ssum[0:64, 0:1])

                def exp1_tp0():
                    expw = state["expw"]
                    expw1 = spool.tile([64, NK], BF16, tag="expw1")
                    ssum1 = spool.tile([64, 1], F32, tag="ssum1")
                    state["expw1"], state["ssum1"] = expw1, ssum1
                    nc.scalar.activation(expw1[:], dists[1][:], ACT.Exp,
                                         bias=0.0, scale=-1.0,
                                         accum_out=ssum1[:, 0:1])
                    # transpose on the tensor engine (psum) + DVE copies:
                    # cheaper and conflict-free vs the xbar DMA path
                    expt = spool.tile([128, 8 * 128], BF16, tag="expt")
                    state["expt"] = expt
                    for jt in range(8):
                        tp = opool.tile([128, 64], BF16, tag="outp")
                        nc.tensor.transpose(tp[:], expw[0:64, jt * 128:(jt + 1) * 128],
                                            ident_sb[0:64, 0:64])
                        nc.vector.tensor_copy(
                            expt[:, jt * 128:jt * 128 + 64], tp[:])

                def tp1():
                    expw1, ssum, ssum1 = state["expw1"], state["ssum"], state["ssum1"]
                    expt = state["expt"]
                    nc.gpsimd.dma_start(out=sout[rr, 0:64], in_=ssum[0:64, 0])
                    nc.gpsimd.dma_start(out=sout[rr, 64:128], in_=ssum1[:, 0])
                    for jt in range(8):
                        tp = opool.tile([128, 64], BF16, tag="outp")
                        nc.tensor.transpose(tp[:], expw1[:, jt * 128:(jt + 1) * 128],
                                            ident_sb[0:64, 0:64])
                        nc.vector.tensor_copy(
                            expt[:, jt * 128 + 64:jt * 128 + 128], tp[:])

                def value():
                    expt = state["expt"]
                    out_ps = opool.tile([D, 128], F32, tag="outp")
                    for jt in range(8):
                        nc.tensor.matmul(out_ps[:, :],
                                         r_sb[:, jt * D:(jt + 1) * D],
                                         expt[:, jt * 128:(jt + 1) * 128],
                                         start=(jt == 0), stop=(jt == 7),
                                         skip_group_check=True)
                    ot = ot_sbs[rr]
                    nc.scalar.copy(ot[:], out_ps[:])
                    nc.sync.dma_start(out=yout[rr, :, :], in_=ot[:])
                return exp0, exp1_tp0, tp1, value

            prev = None
            for rr in range(2):
                ot_sbs[rr] = otpool.tile([D, 128], F32, name="ot", tag="ot")
                dists = []
                for g in range(2):
                    dist = dpool.tile([64, NK], F32, name="dist", tag="dist")
                    dists.append(dist)
                    for s in range(32):
                        p = rr * 64 + g * 32 + s
                        mt = mpool.tile([128, NK], F16, tag="mt")
                        if _is_act_pair(p):
                            nc.scalar.activation(mt[:], kt2_sb[:], ACT.Abs,
                                                 bias=q2t_sb[:, p:p + 1], scale=-1.0)
                            bi = s + 32
                        elif p < 8:
                            # per-window halves: lets window-0 matmuls start
                            # before the second kt2 window is computed
                            for w in range(NWIN):
                                nc.vector.tensor_scalar(
                                    mt[:, w * 512:(w + 1) * 512],
                                    kt2_sb[:, w * 512:(w + 1) * 512],
                                    q2t_sb[:, p:p + 1], None, ALU.min)
                            bi = s
                        else:
                            nc.vector.tensor_scalar(mt[:], kt2_sb[:],
                                                    q2t_sb[:, p:p + 1], None, ALU.min)
                            bi = s
                        for w in range(NWIN):
                            nc.tensor.matmul(
                                dist[:, w * 512:(w + 1) * 512],
                                wones_sb[:, bi * 64:(bi + 1) * 64],
                                mt[:, w * 512:(w + 1) * 512],
                                start=(s == 0), stop=False, skip_group_check=True)
                        if g == 0 and prev is not None:
                            if s == 4:
                                prev[2]()      # g1-half transposes of prev round
                            elif s == 16:
                                prev[3]()      # value matmuls of prev round
                                prev = None
                    cm0 = 0 if (rr == 0 and g == 0) else 64
                    for w in range(NWIN):
                        nc.tensor.matmul(
                            dist[:, w * 512:(w + 1) * 512],
                            cmask_sb[:, cm0:cm0 + 64],
                            arow_sb[:, w * 512:(w + 1) * 512],
                            start=False, stop=True, skip_group_check=True)
                    cur = make_tail(rr, dists, final=(rr == 1)) if g == 0 else cur
                    if g == 0:
                        cur[0]()               # exp of g0
                    else:
                        cur[1]()               # exp g1 + g0-half transposes
                        prev = cur
            prev[2]()
            prev[3]()

    _split_excess_waits(nc)
    return nc


_NC_CACHE = None


def _get_nc():
    global _NC_CACHE
    if _NC_CACHE is None:
        _NC_CACHE = _build_program()
    return _NC_CACHE


def kernel(x1, x2, r, Wk1, bk1, Wk2, bk2, Wq1, bq1, Wq2, bq2):
    global LAST_RESULT
    x1 = np.asarray(x1, np.float32)
    x2 = np.asarray(x2, np.float32)
    r = np.asarray(r, np.float32)
    Wk1 = np.asarray(Wk1, np.float32); bk1 = np.asarray(bk1, np.float32)
    Wk2 = np.asarray(Wk2, np.float32); bk2 = np.asarray(bk2, np.float32)
    Wq1 = np.asarray(Wq1, np.float32); bq1 = np.asarray(bq1, np.float32)
    Wq2 = np.asarray(Wq2, np.float32); bq2 = np.asarray(bq2, np.float32)

    # constant PE weights: ones-block lhsT; blocks 0-31 carry coefficient -2
    # (min-form), blocks 32-63 carry +1 (abs-form). Column block s covers psum
    # rows (2s, 2s+1). cmask: A_j-correction masks (group0 | later groups).
    wones = np.zeros((128, 64 * 64), np.float32)
    cmask = np.zeros((1, 192), np.float32)
    for s in range(32):
        wones[0:64, s * 64 + 2 * s] = -2.0
        wones[64:128, s * 64 + 2 * s + 1] = -2.0
        wones[0:64, (s + 32) * 64 + 2 * s] = 1.0
        wones[64:128, (s + 32) * 64 + 2 * s + 1] = 1.0
        if not _is_act_pair(s):            # group 0 (p = s)
            cmask[0, 2 * s] = 1.0
            cmask[0, 2 * s + 1] = 1.0
        if not _is_act_pair(32 + s):       # groups 1-3
            cmask[0, 64 + 2 * s] = 1.0
            cmask[0, 64 + 2 * s + 1] = 1.0
    shared = {
        "wk1": Wk1.astype(np.float16), "bk1": bk1.reshape(D, 1),
        "wk2d": np.concatenate([Wk2, Wk2], axis=1).astype(np.float16),
        "bk2d": np.concatenate([bk2, bk2]).reshape(128, 1),
        "wq1": Wq1.astype(np.float16), "bq1": bq1.reshape(D, 1),
        "wq2": Wq2.astype(np.float16),
        "bq2d": np.concatenate([bq2, bq2]).reshape(128, 1),
        "wones": wones.astype(np.float16), "cmask": cmask.astype(np.float16),
        "ones64": np.ones((D, 1), np.float16),
        "ident": np.eye(128, dtype=ml_dtypes.bfloat16),
    }
    shared = {k: np.ascontiguousarray(v) for k, v in shared.items()}

    in_maps = []
    for c in range(NCORES):
        b, h = c // 2, c % 2
        m = dict(shared)
        m["x1t"] = np.ascontiguousarray(x1[b].T.astype(np.float16))
        m["x2t"] = np.ascontiguousarray(x2[b, h * QSH:(h + 1) * QSH].T.astype(np.float16))
        m["rv"] = np.ascontiguousarray(r[b].astype(ml_dtypes.bfloat16))
        in_maps.append(m)

    nc = _get_nc()
    trace = bool(os.environ.get("BASS_TRACE"))
    if trace:
        _install_ntff_shim()
    res = None
    for attempt in range(3):
        try:
            res = bass_utils.run_bass_kernel_spmd(
                nc, in_maps, core_ids=list(range(NCORES)), trace=trace)
            break
        except Exception:
            # transient NRT_EXEC_UNIT_UNRECOVERABLE failures have been
            # observed on this fabric; retry (compile results are cached)
            if attempt == 2:
                raise
            import time
            time.sleep(5)
    LAST_RESULT = res

    # reassemble: yout[r, f, t] with t = g*64 + m, m = 2s + i2,
    # local query = i2*128 + r*64 + g*32 + s
    t = np.arange(128)
    g = t // 64
    m = t % 64
    s = m // 2
    i2 = m % 2
    out = np.empty((B, NQ, D), np.float32)
    for c in range(NCORES):
        b, h = c // 2, c % 2
        yc = res.results[c]["yout"]          # [2, D, 128]
        sc = res.results[c]["sout"]          # [2, 128]
        for rr in range(2):
            qloc = i2 * 128 + rr * 64 + g * 32 + s
            out[b, h * QSH + qloc, :] = (yc[rr] / sc[rr][None, :]).T
    return out



# revision 16
# speedup vs baseline: 1.1052x; 1.1052x over previous
"""Laplace attention kernel for Trainium2, 8 NeuronCores.

Math (per batch b):
  k = MLP_k(x1[b])  [NK, D];  q = MLP_q(x2[b])  [NQ, D]
  dist[i,j] = sum_d |k[j,d] - q[i,d]|
  out = softmax_j(-dist) @ r[b]

Distribution: core c = (b, h) = (c//2, c%2): batch b, query-half h (256 queries).

Per-core algorithm (v2 — direct-abs form):
  - MLPs run transposed on the PE: kT2 [128=(i2,d), NK] holds kT stacked twice,
    q2T [128=(i2,d), 128] holds qT for query pairs (p, p+128).
  - For each query pair p a [128, NK] tile Mt = |kT2 - q_p| is produced either
    on the DVE (chained tensor_scalar: (k - q) abs_max 0, 4x f16 mode) or on
    the ACT engine (activation Abs, bias=q, scale=-1).
  - One PE matmul per 512-column window reduces the 128 partitions to the
    pair's two psum rows with a shared constant [128, 2] ones lhsT
    (start/stop per pair, rows 2s/2s+1 of a [128, NK] round tile).
    No correction terms needed: psum holds dist directly (B_i-free).
  - softmax: one ACT Exp per round (scale=-1) over [128, NK] with accum_out
    row-sums; no max-subtraction needed since dist >= 0 and min dist < 80.
  - value matmul: PE transposes of the bf16 weights into psum, two strided
    DVE copies to SBUF, then 8 accumulating PE matmuls against r blocks.
"""

import os
import numpy as np
import ml_dtypes

import concourse.bass as bass
import concourse.mybir as mybir
from concourse.tile import TileContext
from concourse import bass_utils

B, NQ, NK, D = 4, 512, 1024, 64
NCORES = 8
QSH = NQ // 2           # queries per core
NPAIR = QSH // 2        # 128 query pairs per core
NWIN = NK // 512        # 512-column matmul windows

F32 = mybir.dt.float32
F16 = mybir.dt.float16
BF16 = mybir.dt.bfloat16

LAST_RESULT = None      # BassKernelResults of the most recent run (for test.py)

# pairs produced on ACT instead of DVE: late in each 32-pair group so the
# ACT engine can clear MLP evacuations / exps first
ACT_SLOTS = (13, 15, 18, 20, 23, 25, 28, 30)


def _is_act_pair(p):
    return (p % 32) in ACT_SLOTS


# ---------------------------------------------------------------------------
# walrus workaround: the CTRL-class instructions (Drain etc.) can carry only a
# few sem waits; hoist excess waits onto injected NoOps on the same engine.
def _split_excess_waits(nc, max_waits=1):
    for f in nc.m.functions:
        for bb in f.blocks:
            new_insts = []
            for inst in bb.instructions:
                si = inst.sync_info
                if si is not None and si.on_wait and len(si.on_wait) > max_waits:
                    waits = list(si.on_wait)
                    excess, keep = waits[:-max_waits], waits[-max_waits:]
                    for i in range(0, len(excess), max_waits):
                        nop = mybir.InstNoOp(
                            name=f"{inst.name}_waitsplit_{i // max_waits}",
                            ins=[], outs=[])
                        nop.engine = inst.engine
                        nop.sync_info = mybir.SyncInfo(
                            on_wait=excess[i:i + max_waits], on_update=[])
                        new_insts.append(nop)
                    si.on_wait = keep
                new_insts.append(inst)
            bb.instructions = new_insts


# shim antenv.axon_hooks (absent in this image) so BASS_TRACE=1 profiling works
def _install_ntff_shim():
    import sys, types
    if 'antenv.axon_hooks' in sys.modules:
        return
    try:
        mod = types.ModuleType('antenv.axon_hooks')
        state = {}
        mod.set_axon_ntff_profile_hook = lambda h: state.__setitem__('h', h)
        mod.get_axon_ntff_profile_hook = lambda: state.get('h')
        sys.modules['antenv.axon_hooks'] = mod
        import antenv
        antenv.axon_hooks = mod
        from trn_agent_boot.trn_boot import _ntff_profile_via_ctypes
        h = _ntff_profile_via_ctypes('/opt/axon/libaxon_pjrt.so')
        if h is not None:
            mod.set_axon_ntff_profile_hook(h)
    except Exception:
        pass


# ---------------------------------------------------------------------------
def _build_program():
    nc = bass.Bass("TRN2")

    ALU = mybir.AluOpType
    ACT = mybir.ActivationFunctionType

    x1t = nc.dram_tensor("x1t", [D, NK], F16, kind="ExternalInput")
    x2t = nc.dram_tensor("x2t", [D, QSH], F16, kind="ExternalInput")
    rv8 = nc.dram_tensor("rv8", [128, 8 * D], BF16, kind="ExternalInput")
    # packed f16 weights: wq1 | wq2 | wk1 | wk2d | ones  -> [64, 321]
    wpack = nc.dram_tensor("wpack", [D, 321], F16, kind="ExternalInput")
    # packed f32 biases: col0 = [bq1; bk1], col1 = bq2d, col2 = bk2d
    bpack = nc.dram_tensor("bpack", [128, 3], F32, kind="ExternalInput")
    # lhsT stripe blocks: 16 variants of [128, 32]: block m writes psum rows
    # 2m (partitions 0:64) / 2m+1 (partitions 64:128) of a [32, *] region
    # (base partition must be 0/32/64).  [0:512] = min-form (-2 coeff),
    # [512:1024] = abs-form (+1).  [1024:1088] row 0 = cmask (1 at min rows).
    labs = nc.dram_tensor("labs", [128, 1088], F16, kind="ExternalInput")
    ident = nc.dram_tensor("ident", [D, D], BF16, kind="ExternalInput")
    yout = nc.dram_tensor("yout", [2, D, 128], F32, kind="ExternalOutput")
    sout = nc.dram_tensor("sout", [2, 128], F32, kind="ExternalOutput")

    with TileContext(nc) as tc:
        import contextlib
        with contextlib.ExitStack() as ctx:
            consts = ctx.enter_context(tc.tile_pool(name="consts", bufs=1))

            x1t_sb = consts.tile([D, NK], F16)
            x2t_sb = consts.tile([D, QSH], F16)
            r_sb = consts.tile([128, 8 * D], BF16)
            wpack_sb = consts.tile([D, 321], F16)
            bpack_sb = consts.tile([128, 3], F32)
            labs_sb = consts.tile([128, 1088], F16)
            ident_sb = consts.tile([D, D], BF16)

            wq1_sb = wpack_sb[:, 0:64]
            wq2_sb = wpack_sb[:, 64:128]
            wk1_sb = wpack_sb[:, 128:192]
            wk2d_sb = wpack_sb[:, 192:320]
            ones64_sb = wpack_sb[:, 320:321]
            bq1_ap = bpack_sb[0:64, 0:1]
            bk1_ap = bpack_sb[64:128, 0:1]
            bq2d_ap = bpack_sb[:, 1:2]
            bk2d_ap = bpack_sb[:, 2:3]

            # DMA issue order is the schedule: sync carries the q-path (the
            # first matmuls), vector carries x1t, gpsimd carries r.
            nc.sync.dma_start(out=x2t_sb[:], in_=x2t[:, :])
            nc.sync.dma_start(out=wpack_sb[:], in_=wpack[:, :])
            nc.sync.dma_start(out=bpack_sb[:], in_=bpack[:, :])
            nc.sync.dma_start(out=labs_sb[:], in_=labs[:, :])
            nc.sync.dma_start(out=ident_sb[:], in_=ident[:, :])
            nc.gpsimd.dma_start(out=x1t_sb[:, 0:512], in_=x1t[:, 0:512])
            nc.gpsimd.dma_start(out=x1t_sb[:, 512:1024], in_=x1t[:, 512:1024])
            nc.gpsimd.dma_start(out=r_sb[:], in_=rv8[:, :])

            kt2_sb = consts.tile([128, NK], F16)
            q2t_sb = consts.tile([128, 128], F32)
            ht_sb = consts.tile([D, NK], F16)
            hqt_sb = consts.tile([D, QSH], F16)
            arow_sb = consts.tile([1, NK], F16)

            # ---- MLPs (transposed) ----
            with tc.tile_pool(name="mlppsum", bufs=2, space="PSUM") as mp:
                phq = mp.tile([D, QSH], F32, tag="ph")
                nc.tensor.matmul(phq[:], wq1_sb, x2t_sb[:], start=True, stop=True)
                nc.scalar.activation(hqt_sb[:], phq[:], ACT.Relu,
                                     bias=bq1_ap, scale=1.0)
                pq = mp.tile([128, 128], F32, tag="pk")
                nc.tensor.matmul(pq[0:64, :], wq2_sb, hqt_sb[:, 0:128],
                                 start=True, stop=False, skip_group_check=True)
                nc.tensor.matmul(pq[64:128, :], wq2_sb, hqt_sb[:, 128:256],
                                 start=True, stop=True, skip_group_check=True)
                nc.scalar.activation(q2t_sb[:], pq[:], ACT.Identity,
                                     bias=bq2d_ap, scale=1.0)
                for w in range(NWIN):
                    ph = mp.tile([D, 512], F32, tag="ph")
                    nc.tensor.matmul(ph[:], wk1_sb, x1t_sb[:, w * 512:(w + 1) * 512],
                                     start=True, stop=True)
                    nc.scalar.activation(ht_sb[:, w * 512:(w + 1) * 512], ph[:],
                                         ACT.Relu, bias=bk1_ap, scale=1.0)
                    pk = mp.tile([128, 512], F32, tag="pk")
                    nc.tensor.matmul(pk[:], wk2d_sb, ht_sb[:, w * 512:(w + 1) * 512],
                                     start=True, stop=True)
                    nc.scalar.activation(kt2_sb[:, w * 512:(w + 1) * 512], pk[:],
                                         ACT.Identity, bias=bk2d_ap, scale=1.0)
                # A_j = sum_d k[j, d] from the same f16 kt2 the min path sees
                pa = mp.tile([1, NK], F32, tag="pa")
                for w in range(NWIN):
                    nc.tensor.matmul(pa[:, w * 512:(w + 1) * 512], ones64_sb,
                                     kt2_sb[0:64, w * 512:(w + 1) * 512],
                                     start=True, stop=True, skip_group_check=True)
                nc.vector.tensor_copy(arow_sb[:], pa[:])

            # ---- main loop ----
            mpool = ctx.enter_context(tc.tile_pool(name="mtiles", bufs=6))
            dpool = ctx.enter_context(
                tc.tile_pool(name="dist", bufs=2, space="PSUM"))
            opool = ctx.enter_context(
                tc.tile_pool(name="outp", bufs=2, space="PSUM"))
            vpool = ctx.enter_context(
                tc.tile_pool(name="valp", bufs=2, space="PSUM"))
            spool = ctx.enter_context(tc.tile_pool(name="smax", bufs=2))
            otpool = ctx.enter_context(tc.tile_pool(name="outs", bufs=2))

            def make_tail(rr, dists):
                state = {"expw": [None, None], "ssum": [None, None]}

                def expf(g):
                    expw = spool.tile([64, NK], BF16, tag=f"expw{g}")
                    ssum = spool.tile([64, 1], F32, tag=f"ssum{g}")
                    state["expw"][g], state["ssum"][g] = expw, ssum
                    nc.scalar.activation(expw[:], dists[g][:], ACT.Exp,
                                         bias=0.0, scale=-1.0,
                                         accum_out=ssum[:, 0:1])
                    nc.gpsimd.dma_start(out=sout[rr, g * 64:(g + 1) * 64],
                                        in_=ssum[:, 0])

                def transp():
                    expt = spool.tile([128, 8, 128], BF16, tag="expt")
                    state["expt"] = expt
                    for g in range(2):
                        expw = state["expw"][g]
                        tp = opool.tile([128, 8 * D], BF16, tag="outp")
                        for jt in range(8):
                            nc.tensor.transpose(
                                tp[:, jt * D:(jt + 1) * D],
                                expw[:, jt * 128:(jt + 1) * 128],
                                ident_sb[:])
                        nc.vector.tensor_copy(
                            expt[:, :, g * 64:(g + 1) * 64], tp[:])

                def value():
                    expt = state["expt"]
                    out_ps = vpool.tile([D, 128], F32, tag="vout")
                    for jt in range(8):
                        nc.tensor.matmul(out_ps[:, :],
                                         r_sb[:, jt * D:(jt + 1) * D],
                                         expt[:, jt, :],
                                         start=(jt == 0), stop=(jt == 7),
                                         skip_group_check=True)
                    ot = otpool.tile([D, 128], F32, tag="ot")
                    nc.scalar.copy(ot[:], out_ps[:])
                    nc.sync.dma_start(out=yout[rr, :, :], in_=ot[:])

                return expf, transp, value

            prev = None
            for rr in range(2):
                dists = []
                cur = None
                for g in range(2):
                    dist = dpool.tile([64, NK], F32, name="dist", tag="dist")
                    dists.append(dist)
                    for s in range(32):
                        p = rr * 64 + g * 32 + s
                        base, m = 32 * (s // 16), s % 16
                        mt = mpool.tile([128, NK], F16, tag="mt")
                        if _is_act_pair(p):
                            nc.scalar.activation(mt[:], kt2_sb[:], ACT.Abs,
                                                 bias=q2t_sb[:, p:p + 1],
                                                 scale=-1.0)
                            lho = 512      # abs-form stripes (+1)
                        elif p < 12:
                            # per-window halves: window-0 matmuls start before
                            # the second kt2 window is computed
                            for w in range(NWIN):
                                nc.vector.tensor_scalar(
                                    mt[:, w * 512:(w + 1) * 512],
                                    kt2_sb[:, w * 512:(w + 1) * 512],
                                    q2t_sb[:, p:p + 1], None, ALU.min)
                            lho = 0        # min-form stripes (-2)
                        else:
                            nc.vector.tensor_scalar(
                                mt[:], kt2_sb[:], q2t_sb[:, p:p + 1], None,
                                ALU.min)
                            lho = 0
                        for w in range(NWIN):
                            nc.tensor.matmul(
                                dist[base:base + 32, w * 512:(w + 1) * 512],
                                labs_sb[:, lho + 32 * m:lho + 32 * (m + 1)],
                                mt[:, w * 512:(w + 1) * 512],
                                start=(m == 0), stop=False,
                                skip_group_check=True)
                        if prev is not None and g == 0:
                            if s == 4:
                                prev[1]()      # transposes of prev round
                            elif s == 16:
                                prev[2]()      # value matmuls of prev round
                                prev = None
                    # A_j correction onto min-form rows (cmask at labs col
                    # 1024), closing both 32-row accumulation regions
                    for w in range(NWIN):
                        nc.tensor.matmul(
                            dist[0:64, w * 512:(w + 1) * 512],
                            labs_sb[0:1, 1024:1088],
                            arow_sb[:, w * 512:(w + 1) * 512],
                            start=False, stop=True, skip_group_check=True)
                    if g == 0:
                        cur = make_tail(rr, dists)
                    cur[0](g)                  # exp of this group
                prev = cur
            prev[1]()
            prev[2]()

    _split_excess_waits(nc)
    return nc


_NC_CACHE = None


def _get_nc():
    global _NC_CACHE
    if _NC_CACHE is None:
        _NC_CACHE = _build_program()
    return _NC_CACHE


def kernel(x1, x2, r, Wk1, bk1, Wk2, bk2, Wq1, bq1, Wq2, bq2):
    global LAST_RESULT
    x1 = np.asarray(x1, np.float32)
    x2 = np.asarray(x2, np.float32)
    r = np.asarray(r, np.float32)
    Wk1 = np.asarray(Wk1, np.float32); bk1 = np.asarray(bk1, np.float32)
    Wk2 = np.asarray(Wk2, np.float32); bk2 = np.asarray(bk2, np.float32)
    Wq1 = np.asarray(Wq1, np.float32); bq1 = np.asarray(bq1, np.float32)
    Wq2 = np.asarray(Wq2, np.float32); bq2 = np.asarray(bq2, np.float32)

    # 16 lhsT stripe variants: block m covers cols [32m, 32m+32) with the
    # coefficient at row 2m (partitions 0:64) / 2m+1 (partitions 64:128).
    # [0:512]: min-form (-2); [512:1024]: abs-form (+1); [1024:1088] row 0:
    # cmask (+1 at rows of min-form pairs, for the A_j correction)
    labs = np.zeros((128, 1088), np.float32)
    for m in range(16):
        labs[0:64, 34 * m] = -2.0
        labs[64:128, 34 * m + 1] = -2.0
        labs[0:64, 512 + 34 * m] = 1.0
        labs[64:128, 512 + 34 * m + 1] = 1.0
    for s in range(32):
        if s not in ACT_SLOTS:
            u = 32 * (s // 16) + 2 * (s % 16)
            labs[0, 1024 + u] = 1.0
            labs[0, 1024 + u + 1] = 1.0
    wpack = np.concatenate(
        [Wq1, Wq2, Wk1, np.concatenate([Wk2, Wk2], axis=1),
         np.ones((D, 1), np.float32)], axis=1)
    bpack = np.stack([np.concatenate([bq1, bk1]),
                      np.concatenate([bq2, bq2]),
                      np.concatenate([bk2, bk2])], axis=1)
    shared = {
        "wpack": wpack.astype(np.float16),
        "bpack": bpack.astype(np.float32),
        "labs": labs.astype(np.float16),
        "ident": np.eye(D, dtype=ml_dtypes.bfloat16),
    }
    shared = {k: np.ascontiguousarray(v) for k, v in shared.items()}

    in_maps = []
    for c in range(NCORES):
        b, h = c // 2, c % 2
        m = dict(shared)
        m["x1t"] = np.ascontiguousarray(x1[b].T.astype(np.float16))
        m["x2t"] = np.ascontiguousarray(
            x2[b, h * QSH:(h + 1) * QSH].T.astype(np.float16))
        m["rv8"] = np.ascontiguousarray(
            r[b].reshape(8, 128, D).transpose(1, 0, 2).reshape(128, 8 * D)
            .astype(ml_dtypes.bfloat16))
        in_maps.append(m)

    nc = _get_nc()
    trace = bool(os.environ.get("BASS_TRACE"))
    if trace:
        _install_ntff_shim()
    res = None
    for attempt in range(3):
        try:
            res = bass_utils.run_bass_kernel_spmd(
                nc, in_maps, core_ids=list(range(NCORES)), trace=trace)
            break
        except Exception:
            # transient NRT_EXEC_UNIT_UNRECOVERABLE failures have been
            # observed on this fabric; retry (compile results are cached)
            if attempt == 2:
                raise
            import time
            time.sleep(5)
    LAST_RESULT = res

    # reassemble: yout[rr, :, c]: c = g*64 + u, u = 32b + 2m + i2 -> pair
    # s = 16b + m, local query i2*128 + rr*64 + g*32 + s  (pair p covers
    # queries p and p+128 via the stacked kT2/q2T layout)
    c_idx = np.arange(128)
    g = c_idx // 64
    u = c_idx % 64
    s = 16 * (u // 32) + (u % 32) // 2
    i2 = u % 2
    out = np.empty((B, NQ, D), np.float32)
    for c in range(NCORES):
        b, h = c // 2, c % 2
        yc = res.results[c]["yout"]          # [2, D, 128]
        sc = res.results[c]["sout"]          # [2, 128]
        for rr in range(2):
            qloc = i2 * 128 + rr * 64 + g * 32 + s
            out[b, h * QSH + qloc, :] = (yc[rr] / sc[rr][None, :]).T
    return out


# revision 25
# speedup vs baseline: 1.1403x; 1.0318x over previous
"""Laplace attention kernel for Trainium2, 8 NeuronCores.

Math (per batch b):
  k = MLP_k(x1[b])  [NK, D];  q = MLP_q(x2[b])  [NQ, D]
  dist[i,j] = sum_d |k[j,d] - q[i,d]|
  out = softmax_j(-dist) @ r[b]

Distribution: core c = (b, h) = (c//2, c%2): batch b, query-half h (256 queries).

Per-core algorithm (v2 — direct-abs form):
  - MLPs run transposed on the PE: kT2 [128=(i2,d), NK] holds kT stacked twice,
    q2T [128=(i2,d), 128] holds qT for query pairs (p, p+128).
  - For each query pair p a [128, NK] tile Mt = |kT2 - q_p| is produced either
    on the DVE (chained tensor_scalar: (k - q) abs_max 0, 4x f16 mode) or on
    the ACT engine (activation Abs, bias=q, scale=-1).
  - One PE matmul per 512-column window reduces the 128 partitions to the
    pair's two psum rows with a shared constant [128, 2] ones lhsT
    (start/stop per pair, rows 2s/2s+1 of a [128, NK] round tile).
    No correction terms needed: psum holds dist directly (B_i-free).
  - softmax: one ACT Exp per round (scale=-1) over [128, NK] with accum_out
    row-sums; no max-subtraction needed since dist >= 0 and min dist < 80.
  - value matmul: PE transposes of the bf16 weights into psum, two strided
    DVE copies to SBUF, then 8 accumulating PE matmuls against r blocks.
"""

import os
import numpy as np
import ml_dtypes

import concourse.bass as bass
import concourse.mybir as mybir
from concourse.tile import TileContext
from concourse import bass_utils

B, NQ, NK, D = 4, 512, 1024, 64
NCORES = 8
QSH = NQ // 2           # queries per core
NPAIR = QSH // 2        # 128 query pairs per core
NWIN = NK // 512        # 512-column matmul windows

F32 = mybir.dt.float32
F16 = mybir.dt.float16
BF16 = mybir.dt.bfloat16

LAST_RESULT = None      # BassKernelResults of the most recent run (for test.py)

# pairs produced on ACT instead of DVE: late in each 32-pair group so the
# ACT engine can clear MLP evacuations / exps first
ACT_SLOTS = (13, 15, 18, 20, 23, 25, 28, 30)


def _is_act_pair(p):
    return (p % 32) in ACT_SLOTS


# ---------------------------------------------------------------------------
# walrus workaround: the CTRL-class instructions (Drain etc.) can carry only a
# few sem waits; hoist excess waits onto injected NoOps on the same engine.
def _split_excess_waits(nc, max_waits=1):
    for f in nc.m.functions:
        for bb in f.blocks:
            new_insts = []
            for inst in bb.instructions:
                si = inst.sync_info
                if si is not None and si.on_wait and len(si.on_wait) > max_waits:
                    waits = list(si.on_wait)
                    excess, keep = waits[:-max_waits], waits[-max_waits:]
                    for i in range(0, len(excess), max_waits):
                        nop = mybir.InstNoOp(
                            name=f"{inst.name}_waitsplit_{i // max_waits}",
                            ins=[], outs=[])
                        nop.engine = inst.engine
                        nop.sync_info = mybir.SyncInfo(
                            on_wait=excess[i:i + max_waits], on_update=[])
                        new_insts.append(nop)
                    si.on_wait = keep
                new_insts.append(inst)
            bb.instructions = new_insts


# shim antenv.axon_hooks (absent in this image) so BASS_TRACE=1 profiling works
def _install_ntff_shim():
    import sys, types
    if 'antenv.axon_hooks' in sys.modules:
        return
    try:
        mod = types.ModuleType('antenv.axon_hooks')
        state = {}
        mod.set_axon_ntff_profile_hook = lambda h: state.__setitem__('h', h)
        mod.get_axon_ntff_profile_hook = lambda: state.get('h')
        sys.modules['antenv.axon_hooks'] = mod
        import antenv
        antenv.axon_hooks = mod
        from trn_agent_boot.trn_boot import _ntff_profile_via_ctypes
        h = _ntff_profile_via_ctypes('/opt/axon/libaxon_pjrt.so')
        if h is not None:
            mod.set_axon_ntff_profile_hook(h)
    except Exception:
        pass


# ---------------------------------------------------------------------------
def _build_program():
    nc = bass.Bass("TRN2")

    ALU = mybir.AluOpType
    ACT = mybir.ActivationFunctionType

    x1t = nc.dram_tensor("x1t", [D, NK], F16, kind="ExternalInput")
    x2t = nc.dram_tensor("x2t", [D, QSH], F16, kind="ExternalInput")
    # r blocks with an appended ones column: value matmul row 64 yields the
    # softmax denominator (no separate row-sum / sout DMA needed)
    rv8 = nc.dram_tensor("rv8", [128, 8 * 65], BF16, kind="ExternalInput")
    # packed f16 weights: wq1 | wq2 | wk1 | wk2d | ones  -> [64, 321]
    wpack = nc.dram_tensor("wpack", [D, 321], F16, kind="ExternalInput")
    # packed f32 biases: col0 = [bq1; bk1], col1 = bq2d, col2 = bk2d
    bpack = nc.dram_tensor("bpack", [128, 3], F32, kind="ExternalInput")
    # lhsT stripe blocks: 16 variants of [128, 32]: block m writes psum rows
    # 2m (partitions 0:64) / 2m+1 (partitions 64:128) of a [32, *] region
    # (base partition must be 0/32/64).  [0:512] = min-form (-2 coeff),
    # [512:1024] = abs-form (+1).  [1024:1088] row 0 = cmask (1 at min rows).
    labs = nc.dram_tensor("labs", [128, 1088], F16, kind="ExternalInput")
    ident = nc.dram_tensor("ident", [D, D], BF16, kind="ExternalInput")
    yout = nc.dram_tensor("yout", [2, 65, 128], F32, kind="ExternalOutput")

    with TileContext(nc) as tc:
        import contextlib
        with contextlib.ExitStack() as ctx:
            consts = ctx.enter_context(tc.tile_pool(name="consts", bufs=1))

            x1t_sb = consts.tile([D, NK], F16)
            x2t_sb = consts.tile([D, QSH], F16)
            r_sb = consts.tile([128, 8 * 65], BF16)
            wpack_sb = consts.tile([D, 321], F16)
            bpack_sb = consts.tile([128, 3], F32)
            labs_sb = consts.tile([128, 1088], F16)
            ident_sb = consts.tile([D, D], BF16)

            wq1_sb = wpack_sb[:, 0:64]
            wq2_sb = wpack_sb[:, 64:128]
            wk1_sb = wpack_sb[:, 128:192]
            wk2d_sb = wpack_sb[:, 192:320]
            ones64_sb = wpack_sb[:, 320:321]
            bq1_ap = bpack_sb[0:64, 0:1]
            bk1_ap = bpack_sb[64:128, 0:1]
            bq2d_ap = bpack_sb[:, 1:2]
            bk2d_ap = bpack_sb[:, 2:3]

            # DMA issue order is the schedule: weights and x2t land first in
            # parallel on separate queues so the MLP matmuls start early
            nc.sync.dma_start(out=wpack_sb[:], in_=wpack[:, :])
            nc.sync.dma_start(out=x2t_sb[:], in_=x2t[:, :])
            nc.sync.dma_start(out=ident_sb[:], in_=ident[:, :])
            nc.scalar.dma_start(out=bpack_sb[:], in_=bpack[:, :])
            nc.scalar.dma_start(out=labs_sb[:], in_=labs[:, :])
            nc.gpsimd.dma_start(out=x1t_sb[:, 0:512], in_=x1t[:, 0:512])
            nc.gpsimd.dma_start(out=x1t_sb[:, 512:1024], in_=x1t[:, 512:1024])
            nc.gpsimd.dma_start(out=r_sb[:], in_=rv8[:, :])

            kt2_sb = consts.tile([128, NK], F16)
            q2t_sb = consts.tile([128, 128], F32)
            ht_sb = consts.tile([D, NK], F16)
            hqt_sb = consts.tile([D, QSH], F16)
            arow_sb = consts.tile([1, NK], F16)

            # ---- MLPs (transposed), k/q interleaved so the PE fills the
            # ACT-evacuation latency bubbles ----
            with tc.tile_pool(name="mlppsum", bufs=1, space="PSUM") as mp:
                ph0 = mp.tile([D, 512], F32, tag="ph")
                nc.tensor.matmul(ph0[:], wk1_sb, x1t_sb[:, 0:512],
                                 start=True, stop=True)
                phq = mp.tile([D, QSH], F32, tag="phq")
                nc.tensor.matmul(phq[:], wq1_sb, x2t_sb[:], start=True, stop=True)
                nc.scalar.activation(ht_sb[:, 0:512], ph0[:],
                                     ACT.Relu, bias=bk1_ap, scale=1.0)
                nc.scalar.activation(hqt_sb[:], phq[:], ACT.Relu,
                                     bias=bq1_ap, scale=1.0)
                pk0 = mp.tile([128, 512], F32, tag="pk")
                nc.tensor.matmul(pk0[:], wk2d_sb, ht_sb[:, 0:512],
                                 start=True, stop=True)
                pq = mp.tile([128, 128], F32, tag="pq")
                nc.tensor.matmul(pq[0:64, :], wq2_sb, hqt_sb[:, 0:128],
                                 start=True, stop=False, skip_group_check=True)
                nc.tensor.matmul(pq[64:128, :], wq2_sb, hqt_sb[:, 128:256],
                                 start=True, stop=True, skip_group_check=True)
                nc.scalar.activation(kt2_sb[:, 0:512], pk0[:],
                                     ACT.Identity, bias=bk2d_ap, scale=1.0)
                nc.scalar.activation(q2t_sb[:], pq[:], ACT.Identity,
                                     bias=bq2d_ap, scale=1.0)
                ph1 = mp.tile([D, 512], F32, tag="ph")
                nc.tensor.matmul(ph1[:], wk1_sb, x1t_sb[:, 512:1024],
                                 start=True, stop=True)
                nc.scalar.activation(ht_sb[:, 512:1024], ph1[:],
                                     ACT.Relu, bias=bk1_ap, scale=1.0)
                pk1 = mp.tile([128, 512], F32, tag="pk")
                nc.tensor.matmul(pk1[:], wk2d_sb, ht_sb[:, 512:1024],
                                 start=True, stop=True)
                nc.scalar.activation(kt2_sb[:, 512:1024], pk1[:],
                                     ACT.Identity, bias=bk2d_ap, scale=1.0)
                # A_j = sum_d k[j, d] from the same f16 kt2 the min path sees
                pa = mp.tile([1, NK], F32, tag="pa")
                for w in range(NWIN):
                    nc.tensor.matmul(pa[:, w * 512:(w + 1) * 512], ones64_sb,
                                     kt2_sb[0:64, w * 512:(w + 1) * 512],
                                     start=True, stop=True, skip_group_check=True)
                nc.vector.tensor_copy(arow_sb[:], pa[:])

            # ---- main loop ----
            mpool = ctx.enter_context(tc.tile_pool(name="mtiles", bufs=6))
            dpool = ctx.enter_context(
                tc.tile_pool(name="dist", bufs=2, space="PSUM"))
            opool = ctx.enter_context(
                tc.tile_pool(name="outp", bufs=2, space="PSUM"))
            vpool = ctx.enter_context(
                tc.tile_pool(name="valp", bufs=2, space="PSUM"))
            spool = ctx.enter_context(tc.tile_pool(name="smax", bufs=2))
            otpool = ctx.enter_context(tc.tile_pool(name="outs", bufs=2))

            def make_tail(rr, dists):
                state = {"expw": [None, None]}

                def expf(g):
                    expw = spool.tile([64, NK], BF16, tag=f"expw{g}")
                    state["expw"][g] = expw
                    nc.scalar.activation(expw[:], dists[g][:], ACT.Exp,
                                         bias=0.0, scale=-1.0)

                def transp():
                    expt = spool.tile([128, 8, 128], BF16, tag="expt")
                    state["expt"] = expt
                    for g in range(2):
                        expw = state["expw"][g]
                        tp = opool.tile([128, 8 * D], BF16, tag="outp")
                        for jt in range(8):
                            nc.tensor.transpose(
                                tp[:, jt * D:(jt + 1) * D],
                                expw[:, jt * 128:(jt + 1) * 128],
                                ident_sb[:])
                        nc.vector.tensor_copy(
                            expt[:, :, g * 64:(g + 1) * 64], tp[:])

                def value():
                    expt = state["expt"]
                    out_ps = vpool.tile([65, 128], F32, tag="vout")
                    for jt in range(8):
                        nc.tensor.matmul(out_ps[:, :],
                                         r_sb[:, jt * 65:(jt + 1) * 65],
                                         expt[:, jt, :],
                                         start=(jt == 0), stop=(jt == 7),
                                         skip_group_check=True)
                    ot = otpool.tile([65, 128], F32, tag="ot")
                    nc.scalar.copy(ot[:], out_ps[:])
                    nc.sync.dma_start(out=yout[rr, :, :], in_=ot[:])

                return expf, transp, value

            prev = None
            for rr in range(2):
                dists = []
                cur = None
                for g in range(2):
                    dist = dpool.tile([64, NK], F32, name="dist", tag="dist")
                    dists.append(dist)
                    for s in range(32):
                        p = rr * 64 + g * 32 + s
                        base, m = 32 * (s // 16), s % 16
                        mt = mpool.tile([128, NK], F16, tag="mt")
                        if _is_act_pair(p):
                            nc.scalar.activation(mt[:], kt2_sb[:], ACT.Abs,
                                                 bias=q2t_sb[:, p:p + 1],
                                                 scale=-1.0)
                            lho = 512      # abs-form stripes (+1)
                        elif p < 12:
                            # per-window halves: window-0 matmuls start before
                            # the second kt2 window is computed
                            for w in range(NWIN):
                                nc.vector.tensor_scalar(
                                    mt[:, w * 512:(w + 1) * 512],
                                    kt2_sb[:, w * 512:(w + 1) * 512],
                                    q2t_sb[:, p:p + 1], None, ALU.min)
                            lho = 0        # min-form stripes (-2)
                        else:
                            nc.vector.tensor_scalar(
                                mt[:], kt2_sb[:], q2t_sb[:, p:p + 1], None,
                                ALU.min)
                            lho = 0
                        for w in range(NWIN):
                            nc.tensor.matmul(
                                dist[base:base + 32, w * 512:(w + 1) * 512],
                                labs_sb[:, lho + 32 * m:lho + 32 * (m + 1)],
                                mt[:, w * 512:(w + 1) * 512],
                                start=(m == 0), stop=False,
                                skip_group_check=True)
                        if prev is not None and g == 0:
                            if s == 4:
                                prev[1]()      # transposes of prev round
                            elif s == 16:
                                prev[2]()      # value matmuls of prev round
                                prev = None
                    # A_j correction onto min-form rows (cmask at labs col
                    # 1024), closing both 32-row accumulation regions
                    for w in range(NWIN):
                        nc.tensor.matmul(
                            dist[0:64, w * 512:(w + 1) * 512],
                            labs_sb[0:1, 1024:1088],
                            arow_sb[:, w * 512:(w + 1) * 512],
                            start=False, stop=True, skip_group_check=True)
                    if g == 0:
                        cur = make_tail(rr, dists)
                    cur[0](g)                  # exp of this group
                prev = cur
            prev[1]()
            prev[2]()

    _split_excess_waits(nc)
    return nc


_NC_CACHE = None


def _get_nc():
    global _NC_CACHE
    if _NC_CACHE is None:
        _NC_CACHE = _build_program()
    return _NC_CACHE


def kernel(x1, x2, r, Wk1, bk1, Wk2, bk2, Wq1, bq1, Wq2, bq2):
    global LAST_RESULT
    x1 = np.asarray(x1, np.float32)
    x2 = np.asarray(x2, np.float32)
    r = np.asarray(r, np.float32)
    Wk1 = np.asarray(Wk1, np.float32); bk1 = np.asarray(bk1, np.float32)
    Wk2 = np.asarray(Wk2, np.float32); bk2 = np.asarray(bk2, np.float32)
    Wq1 = np.asarray(Wq1, np.float32); bq1 = np.asarray(bq1, np.float32)
    Wq2 = np.asarray(Wq2, np.float32); bq2 = np.asarray(bq2, np.float32)

    # 16 lhsT stripe variants: block m covers cols [32m, 32m+32) with the
    # coefficient at row 2m (partitions 0:64) / 2m+1 (partitions 64:128).
    # [0:512]: min-form (-2); [512:1024]: abs-form (+1); [1024:1088] row 0:
    # cmask (+1 at rows of min-form pairs, for the A_j correction)
    labs = np.zeros((128, 1088), np.float32)
    for m in range(16):
        labs[0:64, 34 * m] = -2.0
        labs[64:128, 34 * m + 1] = -2.0
        labs[0:64, 512 + 34 * m] = 1.0
        labs[64:128, 512 + 34 * m + 1] = 1.0
    for s in range(32):
        if s not in ACT_SLOTS:
            u = 32 * (s // 16) + 2 * (s % 16)
            labs[0, 1024 + u] = 1.0
            labs[0, 1024 + u + 1] = 1.0
    wpack = np.concatenate(
        [Wq1, Wq2, Wk1, np.concatenate([Wk2, Wk2], axis=1),
         np.ones((D, 1), np.float32)], axis=1)
    bpack = np.stack([np.concatenate([bq1, bk1]),
                      np.concatenate([bq2, bq2]),
                      np.concatenate([bk2, bk2])], axis=1)
    shared = {
        "wpack": wpack.astype(np.float16),
        "bpack": bpack.astype(np.float32),
        "labs": labs.astype(np.float16),
        "ident": np.eye(D, dtype=ml_dtypes.bfloat16),
    }
    shared = {k: np.ascontiguousarray(v) for k, v in shared.items()}

    in_maps = []
    for c in range(NCORES):
        b, h = c // 2, c % 2
        m = dict(shared)
        m["x1t"] = np.ascontiguousarray(x1[b].T.astype(np.float16))
        m["x2t"] = np.ascontiguousarray(
            x2[b, h * QSH:(h + 1) * QSH].T.astype(np.float16))
        rb = r[b].reshape(8, 128, D).transpose(1, 0, 2)     # [128, 8, 64]
        rb = np.concatenate(
            [rb, np.ones((128, 8, 1), np.float32)], axis=2)  # ones col
        m["rv8"] = np.ascontiguousarray(
            rb.reshape(128, 8 * 65).astype(ml_dtypes.bfloat16))
        in_maps.append(m)

    nc = _get_nc()
    trace = bool(os.environ.get("BASS_TRACE"))
    if trace:
        _install_ntff_shim()
    res = None
    for attempt in range(3):
        try:
            res = bass_utils.run_bass_kernel_spmd(
                nc, in_maps, core_ids=list(range(NCORES)), trace=trace)
            break
        except Exception:
            # transient NRT_EXEC_UNIT_UNRECOVERABLE failures have been
            # observed on this fabric; retry (compile results are cached)
            if attempt == 2:
                raise
            import time
            time.sleep(5)
    LAST_RESULT = res

    # reassemble: yout[rr, :, c]: c = g*64 + u, u = 32b + 2m + i2 -> pair
    # s = 16b + m, local query i2*128 + rr*64 + g*32 + s  (pair p covers
    # queries p and p+128 via the stacked kT2/q2T layout)
    c_idx = np.arange(128)
    g = c_idx // 64
    u = c_idx % 64
    s = 16 * (u // 32) + (u % 32) // 2
    i2 = u % 2
    out = np.empty((B, NQ, D), np.float32)
    for c in range(NCORES):
        b, h = c // 2, c % 2
        yc = res.results[c]["yout"]          # [2, 65, 128]
        for rr in range(2):
            qloc = i2 * 128 + rr * 64 + g * 32 + s
            out[b, h * QSH + qloc, :] = (yc[rr, 0:64] / yc[rr, 64][None, :]).T
    return out


# revision 34
# speedup vs baseline: 1.1704x; 1.0264x over previous
"""Laplace attention kernel for Trainium2, 8 NeuronCores.

Math (per batch b):
  k = MLP_k(x1[b])  [NK, D];  q = MLP_q(x2[b])  [NQ, D]
  dist[i,j] = sum_d |k[j,d] - q[i,d]|
  out = softmax_j(-dist) @ r[b]

Distribution: core c = (b, h) = (c//2, c%2): batch b, query-half h (256 queries).

Per-core algorithm (v2 — direct-abs form):
  - MLPs run transposed on the PE: kT2 [128=(i2,d), NK] holds kT stacked twice,
    q2T [128=(i2,d), 128] holds qT for query pairs (p, p+128).
  - For each query pair p a [128, NK] tile Mt = |kT2 - q_p| is produced either
    on the DVE (chained tensor_scalar: (k - q) abs_max 0, 4x f16 mode) or on
    the ACT engine (activation Abs, bias=q, scale=-1).
  - One PE matmul per 512-column window reduces the 128 partitions to the
    pair's two psum rows with a shared constant [128, 2] ones lhsT
    (start/stop per pair, rows 2s/2s+1 of a [128, NK] round tile).
    No correction terms needed: psum holds dist directly (B_i-free).
  - softmax: one ACT Exp per round (scale=-1) over [128, NK] with accum_out
    row-sums; no max-subtraction needed since dist >= 0 and min dist < 80.
  - value matmul: PE transposes of the bf16 weights into psum, two strided
    DVE copies to SBUF, then 8 accumulating PE matmuls against r blocks.
"""

import os
import numpy as np
import ml_dtypes

import concourse.bass as bass
import concourse.mybir as mybir
from concourse.tile import TileContext
from concourse import bass_utils

B, NQ, NK, D = 4, 512, 1024, 64
NCORES = 8
QSH = NQ // 2           # queries per core
NPAIR = QSH // 2        # 128 query pairs per core
NWIN = NK // 512        # 512-column matmul windows

F32 = mybir.dt.float32
F16 = mybir.dt.float16
BF16 = mybir.dt.bfloat16

LAST_RESULT = None      # BassKernelResults of the most recent run (for test.py)

# pairs produced on ACT instead of DVE.  In the first section the ACT
# engine is free right after the MLP evacuations, and the DVE is the
# early-pipeline constraint, so ACT starts earlier there.
ACT_SLOTS = (13, 15, 18, 20, 23, 25, 28, 30)
ACT_SLOTS0 = (5, 7, 9, 11, 13, 15, 18, 20)


def _is_act_pair(p):
    s = p % 32
    return s in (ACT_SLOTS0 if p < 32 else ACT_SLOTS)


# ---------------------------------------------------------------------------
# walrus workaround: the CTRL-class instructions (Drain etc.) can carry only a
# few sem waits; hoist excess waits onto injected NoOps on the same engine.
def _split_excess_waits(nc, max_waits=1):
    for f in nc.m.functions:
        for bb in f.blocks:
            new_insts = []
            for inst in bb.instructions:
                si = inst.sync_info
                if si is not None and si.on_wait and len(si.on_wait) > max_waits:
                    waits = list(si.on_wait)
                    excess, keep = waits[:-max_waits], waits[-max_waits:]
                    for i in range(0, len(excess), max_waits):
                        nop = mybir.InstNoOp(
                            name=f"{inst.name}_waitsplit_{i // max_waits}",
                            ins=[], outs=[])
                        nop.engine = inst.engine
                        nop.sync_info = mybir.SyncInfo(
                            on_wait=excess[i:i + max_waits], on_update=[])
                        new_insts.append(nop)
                    si.on_wait = keep
                new_insts.append(inst)
            bb.instructions = new_insts


# shim antenv.axon_hooks (absent in this image) so BASS_TRACE=1 profiling works
def _install_ntff_shim():
    import sys, types
    if 'antenv.axon_hooks' in sys.modules:
        return
    try:
        mod = types.ModuleType('antenv.axon_hooks')
        state = {}
        mod.set_axon_ntff_profile_hook = lambda h: state.__setitem__('h', h)
        mod.get_axon_ntff_profile_hook = lambda: state.get('h')
        sys.modules['antenv.axon_hooks'] = mod
        import antenv
        antenv.axon_hooks = mod
        from trn_agent_boot.trn_boot import _ntff_profile_via_ctypes
        h = _ntff_profile_via_ctypes('/opt/axon/libaxon_pjrt.so')
        if h is not None:
            mod.set_axon_ntff_profile_hook(h)
    except Exception:
        pass


# ---------------------------------------------------------------------------
def _build_program():
    nc = bass.Bass("TRN2")

    ALU = mybir.AluOpType
    ACT = mybir.ActivationFunctionType

    x1t = nc.dram_tensor("x1t", [D, NK], F16, kind="ExternalInput")
    x2t = nc.dram_tensor("x2t", [D, QSH], F16, kind="ExternalInput")
    # r blocks with an appended ones column: value matmul row 64 yields the
    # softmax denominator (no separate row-sum / sout DMA needed)
    rv8 = nc.dram_tensor("rv8", [128, 8 * 65], BF16, kind="ExternalInput")
    # packed f16 weights: wq1 | wq2 | wk1 | wk2d | ones  -> [64, 321]
    wpack = nc.dram_tensor("wpack", [D, 321], F16, kind="ExternalInput")
    # packed f32 biases: col0 = [bq1; bk1], col1 = bq2d, col2 = bk2d
    bpack = nc.dram_tensor("bpack", [128, 3], F32, kind="ExternalInput")
    # lhsT stripe blocks: 16 variants of [128, 32]: block m writes psum rows
    # 2m (partitions 0:64) / 2m+1 (partitions 64:128) of a [32, *] region
    # (base partition must be 0/32/64).  [0:512] = min-form (-2 coeff),
    # [512:1024] = abs-form (+1).  Row 0 of [1024:1088] / [1088:1152] =
    # cmasks (1 at min rows) for section 0 / later sections.
    labs = nc.dram_tensor("labs", [128, 1152], F16, kind="ExternalInput")
    ident = nc.dram_tensor("ident", [D, D], BF16, kind="ExternalInput")
    yout = nc.dram_tensor("yout", [2, 65, 128], F32, kind="ExternalOutput")

    with TileContext(nc) as tc:
        import contextlib
        with contextlib.ExitStack() as ctx:
            consts = ctx.enter_context(tc.tile_pool(name="consts", bufs=1))

            x1t_sb = consts.tile([D, NK], F16)
            x2t_sb = consts.tile([D, QSH], F16)
            r_sb = consts.tile([128, 8 * 65], BF16)
            wpack_sb = consts.tile([D, 321], F16)
            bpack_sb = consts.tile([128, 3], F32)
            labs_sb = consts.tile([128, 1152], F16)
            ident_sb = consts.tile([D, D], BF16)

            wq1_sb = wpack_sb[:, 0:64]
            wq2_sb = wpack_sb[:, 64:128]
            wk1_sb = wpack_sb[:, 128:192]
            wk2d_sb = wpack_sb[:, 192:320]
            ones64_sb = wpack_sb[:, 320:321]
            bq1_ap = bpack_sb[0:64, 0:1]
            bk1_ap = bpack_sb[64:128, 0:1]
            bq2d_ap = bpack_sb[:, 1:2]
            bk2d_ap = bpack_sb[:, 2:3]

            # force the activation-table load to the head of the ACT queue,
            # before any ACT work is otherwise reachable
            dummy_sb = consts.tile([1, 1], F32)
            nc.vector.memset(dummy_sb[:], 0.0)
            nc.scalar.activation(dummy_sb[:], dummy_sb[:], ACT.Relu)

            # warm-up matmuls on zeroed tiles: the PE p-state needs ~3us of
            # continuous execution to reach full clock, so burn the DMA-wait
            # dead time ramping it up
            warm_w = consts.tile([128, 32], F16)
            warm_r = consts.tile([128, 512], F16)
            nc.vector.memset(warm_w[:], 0.0)
            nc.vector.memset(warm_r[:], 0.0)

            # DMA issue order is the schedule: weights and x2t land first in
            # parallel on separate queues so the MLP matmuls start early
            nc.sync.dma_start(out=wpack_sb[:], in_=wpack[:, :])
            nc.sync.dma_start(out=x2t_sb[:], in_=x2t[:, :])
            nc.sync.dma_start(out=x1t_sb[:, 0:512], in_=x1t[:, 0:512])
            nc.sync.dma_start(out=x1t_sb[:, 512:1024], in_=x1t[:, 512:1024])
            nc.scalar.dma_start(out=bpack_sb[:], in_=bpack[:, :])
            nc.scalar.dma_start(out=labs_sb[:], in_=labs[:, :])
            nc.gpsimd.dma_start(out=ident_sb[:], in_=ident[:, :])
            nc.gpsimd.dma_start(out=r_sb[:], in_=rv8[:, :])

            kt2_sb = consts.tile([128, NK], F16)
            q2t_sb = consts.tile([128, 128], F32)
            ht_sb = consts.tile([D, NK], F16)
            hqt_sb = consts.tile([D, QSH], F16)
            arow_sb = consts.tile([1, NK], F16)

            # ---- MLPs (transposed), k/q interleaved so the PE fills the
            # ACT-evacuation latency bubbles ----
            with tc.tile_pool(name="mlppsum", bufs=1, space="PSUM") as mp:
                ph0 = mp.tile([D, 512], F32, tag="ph")
                nc.tensor.matmul(ph0[:], wk1_sb, x1t_sb[:, 0:512],
                                 start=True, stop=True)
                phq = mp.tile([D, QSH], F32, tag="phq")
                nc.tensor.matmul(phq[:], wq1_sb, x2t_sb[:], start=True, stop=True)
                nc.scalar.activation(ht_sb[:, 0:512], ph0[:],
                                     ACT.Relu, bias=bk1_ap, scale=1.0)
                nc.scalar.activation(hqt_sb[:], phq[:], ACT.Relu,
                                     bias=bq1_ap, scale=1.0)
                pk0 = mp.tile([128, 512], F32, tag="pk")
                nc.tensor.matmul(pk0[:], wk2d_sb, ht_sb[:, 0:512],
                                 start=True, stop=True)
                pq = mp.tile([128, 128], F32, tag="pq")
                nc.tensor.matmul(pq[0:64, :], wq2_sb, hqt_sb[:, 0:128],
                                 start=True, stop=False, skip_group_check=True)
                nc.tensor.matmul(pq[64:128, :], wq2_sb, hqt_sb[:, 128:256],
                                 start=True, stop=True, skip_group_check=True)
                nc.scalar.activation(kt2_sb[:, 0:512], pk0[:],
                                     ACT.Identity, bias=bk2d_ap, scale=1.0)
                nc.scalar.activation(q2t_sb[:], pq[:], ACT.Identity,
                                     bias=bq2d_ap, scale=1.0)
                ph1 = mp.tile([D, 512], F32, tag="ph")
                nc.tensor.matmul(ph1[:], wk1_sb, x1t_sb[:, 512:1024],
                                 start=True, stop=True)
                nc.scalar.activation(ht_sb[:, 512:1024], ph1[:],
                                     ACT.Relu, bias=bk1_ap, scale=1.0)
                pk1 = mp.tile([128, 512], F32, tag="pk")
                nc.tensor.matmul(pk1[:], wk2d_sb, ht_sb[:, 512:1024],
                                 start=True, stop=True)
                nc.scalar.activation(kt2_sb[:, 512:1024], pk1[:],
                                     ACT.Identity, bias=bk2d_ap, scale=1.0)
                # A_j = sum_d k[j, d] from the same f16 kt2 the min path sees
                pa = mp.tile([1, NK], F32, tag="pa")
                for w in range(NWIN):
                    nc.tensor.matmul(pa[:, w * 512:(w + 1) * 512], ones64_sb,
                                     kt2_sb[0:64, w * 512:(w + 1) * 512],
                                     start=True, stop=True, skip_group_check=True)
                nc.vector.tensor_copy(arow_sb[:], pa[:])

            # ---- main loop ----
            mpool = ctx.enter_context(tc.tile_pool(name="mtiles", bufs=8))
            dpool = ctx.enter_context(
                tc.tile_pool(name="dist", bufs=2, space="PSUM"))
            opool = ctx.enter_context(
                tc.tile_pool(name="outp", bufs=2, space="PSUM"))
            vpool = ctx.enter_context(
                tc.tile_pool(name="valp", bufs=2, space="PSUM"))
            spool = ctx.enter_context(tc.tile_pool(name="smax", bufs=2))
            otpool = ctx.enter_context(tc.tile_pool(name="outs", bufs=2))

            def make_tail(rr):
                state = {"expw": [None, None], "expt": None}

                def expf(g, dist):
                    expw = spool.tile([64, NK], BF16, tag=f"expw{g}")
                    state["expw"][g] = expw
                    nc.scalar.activation(expw[:], dist[:], ACT.Exp,
                                         bias=0.0, scale=-1.0)

                def transp(g):
                    if state["expt"] is None:
                        expt = spool.tile([128, 8, 128], BF16, tag="expt")
                        state["expt"] = expt
                    expt = state["expt"]
                    expw = state["expw"][g]
                    tp = opool.tile([128, 8 * D], BF16, tag="outp")
                    for jt in range(8):
                        nc.tensor.transpose(
                            tp[:, jt * D:(jt + 1) * D],
                            expw[:, jt * 128:(jt + 1) * 128],
                            ident_sb[:])
                    nc.vector.tensor_copy(
                        expt[:, :, g * 64:(g + 1) * 64], tp[:])

                def value():
                    expt = state["expt"]
                    out_ps = vpool.tile([65, 128], F32, tag="vout")
                    for jt in range(8):
                        nc.tensor.matmul(out_ps[:, :],
                                         r_sb[:, jt * 65:(jt + 1) * 65],
                                         expt[:, jt, :],
                                         start=(jt == 0), stop=(jt == 7),
                                         skip_group_check=True)
                    ot = otpool.tile([65, 128], F32, tag="ot")
                    nc.scalar.copy(ot[:], out_ps[:])
                    nc.sync.dma_start(out=yout[rr, :, :], in_=ot[:])

                return expf, transp, value

            prev = None
            for rr in range(2):
                # the last round runs g=1 first so its exp/transposes overlap
                # the g=0 pair matmuls, shortening the final tail
                gorder = (0, 1) if rr == 0 else (1, 0)
                cur = make_tail(rr)
                for pos, g in enumerate(gorder):
                    dist = dpool.tile([64, NK], F32, name="dist", tag="dist")
                    for s in range(32):
                        p = rr * 64 + g * 32 + s
                        base, m = 32 * (s // 16), s % 16
                        mt = mpool.tile([128, NK], F16, tag="mt")
                        if _is_act_pair(p):
                            nc.scalar.activation(mt[:], kt2_sb[:], ACT.Abs,
                                                 bias=q2t_sb[:, p:p + 1],
                                                 scale=-1.0)
                            lho = 512      # abs-form stripes (+1)
                        elif p < 12:
                            # per-window halves: window-0 matmuls start before
                            # the second kt2 window is computed
                            for w in range(NWIN):
                                nc.vector.tensor_scalar(
                                    mt[:, w * 512:(w + 1) * 512],
                                    kt2_sb[:, w * 512:(w + 1) * 512],
                                    q2t_sb[:, p:p + 1], None, ALU.min)
                            lho = 0        # min-form stripes (-2)
                        else:
                            nc.vector.tensor_scalar(
                                mt[:], kt2_sb[:], q2t_sb[:, p:p + 1], None,
                                ALU.min)
                            lho = 0
                        for w in range(NWIN):
                            nc.tensor.matmul(
                                dist[base:base + 32, w * 512:(w + 1) * 512],
                                labs_sb[:, lho + 32 * m:lho + 32 * (m + 1)],
                                mt[:, w * 512:(w + 1) * 512],
                                start=(m == 0), stop=False,
                                skip_group_check=True)
                        if prev is not None and pos == 0:
                            if s == 4:
                                prev[1](0)     # prev-round transposes
                                prev[1](1)
                            elif s == 16:
                                prev[2]()      # prev-round value matmuls
                                prev = None
                        elif rr == 1 and pos == 1 and s == 4:
                            cur[1](1)          # early transposes of g=1
                    # A_j correction onto min-form rows, closing both 32-row
                    # accumulation regions (cmask variant per section)
                    cm = 1024 if (rr, g) == (0, 0) else 1088
                    for w in range(NWIN):
                        nc.tensor.matmul(
                            dist[0:64, w * 512:(w + 1) * 512],
                            labs_sb[0:1, cm:cm + 64],
                            arow_sb[:, w * 512:(w + 1) * 512],
                            start=False, stop=True, skip_group_check=True)
                    cur[0](g, dist)            # exp of this group
                prev = cur
            prev[1](0)
            prev[2]()

    _split_excess_waits(nc)
    return nc


_NC_CACHE = None


def _get_nc():
    global _NC_CACHE
    if _NC_CACHE is None:
        _NC_CACHE = _build_program()
    return _NC_CACHE


def kernel(x1, x2, r, Wk1, bk1, Wk2, bk2, Wq1, bq1, Wq2, bq2):
    global LAST_RESULT
    x1 = np.asarray(x1, np.float32)
    x2 = np.asarray(x2, np.float32)
    r = np.asarray(r, np.float32)
    Wk1 = np.asarray(Wk1, np.float32); bk1 = np.asarray(bk1, np.float32)
    Wk2 = np.asarray(Wk2, np.float32); bk2 = np.asarray(bk2, np.float32)
    Wq1 = np.asarray(Wq1, np.float32); bq1 = np.asarray(bq1, np.float32)
    Wq2 = np.asarray(Wq2, np.float32); bq2 = np.asarray(bq2, np.float32)

    # 16 lhsT stripe variants: block m covers cols [32m, 32m+32) with the
    # coefficient at row 2m (partitions 0:64) / 2m+1 (partitions 64:128).
    # [0:512]: min-form (-2); [512:1024]: abs-form (+1); [1024:1088] row 0:
    # cmask (+1 at rows of min-form pairs, for the A_j correction)
    labs = np.zeros((128, 1152), np.float32)
    for m in range(16):
        labs[0:64, 34 * m] = -2.0
        labs[64:128, 34 * m + 1] = -2.0
        labs[0:64, 512 + 34 * m] = 1.0
        labs[64:128, 512 + 34 * m + 1] = 1.0
    for s in range(32):
        u = 32 * (s // 16) + 2 * (s % 16)
        if s not in ACT_SLOTS0:
            labs[0, 1024 + u] = 1.0
            labs[0, 1024 + u + 1] = 1.0
        if s not in ACT_SLOTS:
            labs[0, 1088 + u] = 1.0
            labs[0, 1088 + u + 1] = 1.0
    wpack = np.concatenate(
        [Wq1, Wq2, Wk1, np.concatenate([Wk2, Wk2], axis=1),
         np.ones((D, 1), np.float32)], axis=1)
    bpack = np.stack([np.concatenate([bq1, bk1]),
                      np.concatenate([bq2, bq2]),
                      np.concatenate([bk2, bk2])], axis=1)
    shared = {
        "wpack": wpack.astype(np.float16),
        "bpack": bpack.astype(np.float32),
        "labs": labs.astype(np.float16),
        "ident": np.eye(D, dtype=ml_dtypes.bfloat16),
    }
    shared = {k: np.ascontiguousarray(v) for k, v in shared.items()}

    in_maps = []
    for c in range(NCORES):
        b, h = c // 2, c % 2
        m = dict(shared)
        m["x1t"] = np.ascontiguousarray(x1[b].T.astype(np.float16))
        m["x2t"] = np.ascontiguousarray(
            x2[b, h * QSH:(h + 1) * QSH].T.astype(np.float16))
        rb = r[b].reshape(8, 128, D).transpose(1, 0, 2)     # [128, 8, 64]
        rb = np.concatenate(
            [rb, np.ones((128, 8, 1), np.float32)], axis=2)  # ones col
        m["rv8"] = np.ascontiguousarray(
            rb.reshape(128, 8 * 65).astype(ml_dtypes.bfloat16))
        in_maps.append(m)

    nc = _get_nc()
    trace = bool(os.environ.get("BASS_TRACE"))
    if trace:
        _install_ntff_shim()
    res = None
    for attempt in range(3):
        try:
            res = bass_utils.run_bass_kernel_spmd(
                nc, in_maps, core_ids=list(range(NCORES)), trace=trace)
            break
        except Exception:
            # transient NRT_EXEC_UNIT_UNRECOVERABLE failures have been
            # observed on this fabric; retry (compile results are cached)
            if attempt == 2:
                raise
            import time
            time.sleep(5)
    LAST_RESULT = res

    # reassemble: yout[rr, :, c]: c = g*64 + u, u = 32b + 2m + i2 -> pair
    # s = 16b + m, local query i2*128 + rr*64 + g*32 + s  (pair p covers
    # queries p and p+128 via the stacked kT2/q2T layout)
    c_idx = np.arange(128)
    g = c_idx // 64
    u = c_idx % 64
    s = 16 * (u // 32) + (u % 32) // 2
    i2 = u % 2
    out = np.empty((B, NQ, D), np.float32)
    for c in range(NCORES):
        b, h = c // 2, c % 2
        yc = res.results[c]["yout"]          # [2, 65, 128]
        for rr in range(2):
            qloc = i2 * 128 + rr * 64 + g * 32 + s
            out[b, h * QSH + qloc, :] = (yc[rr, 0:64] / yc[rr, 64][None, :]).T
    return out


# revision 51
# speedup vs baseline: 1.2166x; 1.0394x over previous
"""Laplace attention kernel for Trainium2, 8 NeuronCores.

Math (per batch b):
  k = MLP_k(x1[b])  [NK, D];  q = MLP_q(x2[b])  [NQ, D]
  dist[i,j] = sum_d |k[j,d] - q[i,d]|
  out = softmax_j(-dist) @ r[b]

Distribution: core c = (b, h) = (c//2, c%2): batch b, query-half h (256 queries).

Per-core algorithm (relu form):
  dist = B_i - A_j + 2*sum_d relu(k_jd - q_id)   (A = sum_d k, B = sum_d q)
  so exp(-dist) = exp(-2*sum relu) * exp(A_j) * exp(-B_i); the exp(-B_i)
  factor is row-constant and cancels in the softmax normalization, and
  exp(A_j) is folded into the value vectors r on device at startup.

  - MLPs run transposed on the PE: kT2 [128=(i2,d), NK] holds kT stacked
    twice, q2T [128=(i2,d), 128] holds qT for query pairs (p, p+128).
  - For each query pair p a [128, NK] tile Mt = relu(kT2 - q_p) is produced
    either on the DVE (chained tensor_scalar (k - q) max 0, 4x f16 mode) or
    on the ACT engine (activation Relu, bias=-q).
  - One PE matmul per 512-column window reduces the 128 partitions to the
    pair's two psum rows out of a 32-row region (psum write base must be
    0/32/64) using one of 16 shared [128, 32] +1-stripe lhsT blocks; 16
    pairs accumulate per region.
  - softmax numerator: ACT Exp (scale=-2) per 32-pair group -> bf16.
  - value: PE transposes of the weights into psum, strided DVE copies into
    a [128, 8, 128] SBUF tile, then accumulating PE matmuls against the
    exp(A)-scaled r blocks, whose appended ones-column yields the softmax
    denominator as output row 64 (no separate row-sum or its DMA).
  - The PE p-state stays at 1.2 GHz until ~25us from kernel start (fixed
    hardware ramp); warm-up matmuls start the PE during the input DMAs and
    the schedule keeps it gap-free so the slow window wastes nothing.
"""

import os
import numpy as np
import ml_dtypes

import concourse.bass as bass
import concourse.mybir as mybir
from concourse.tile import TileContext
from concourse import bass_utils

B, NQ, NK, D = 4, 512, 1024, 64
NCORES = 8
QSH = NQ // 2           # queries per core
NPAIR = QSH // 2        # 128 query pairs per core
NWIN = NK // 512        # 512-column matmul windows

F32 = mybir.dt.float32
F16 = mybir.dt.float16
BF16 = mybir.dt.bfloat16

LAST_RESULT = None      # BassKernelResults of the most recent run (for test.py)

# pairs produced on ACT instead of DVE.  In the first section the ACT
# engine is free right after the MLP evacuations, and the DVE is the
# early-pipeline constraint, so ACT starts earlier there.
ACT_SLOTS = (13, 15, 18, 20, 23, 25, 28, 30)
ACT_SLOTS0 = (5, 7, 9, 11, 13, 15, 18, 20)


def _is_act_pair(p):
    s = p % 32
    return s in (ACT_SLOTS0 if p < 32 else ACT_SLOTS)


# ---------------------------------------------------------------------------
# walrus workaround: the CTRL-class instructions (Drain etc.) can carry only a
# few sem waits; hoist excess waits onto injected NoOps on the same engine.
def _split_excess_waits(nc, max_waits=1):
    for f in nc.m.functions:
        for bb in f.blocks:
            new_insts = []
            for inst in bb.instructions:
                si = inst.sync_info
                if si is not None and si.on_wait and len(si.on_wait) > max_waits:
                    waits = list(si.on_wait)
                    excess, keep = waits[:-max_waits], waits[-max_waits:]
                    for i in range(0, len(excess), max_waits):
                        nop = mybir.InstNoOp(
                            name=f"{inst.name}_waitsplit_{i // max_waits}",
                            ins=[], outs=[])
                        nop.engine = inst.engine
                        nop.sync_info = mybir.SyncInfo(
                            on_wait=excess[i:i + max_waits], on_update=[])
                        new_insts.append(nop)
                    si.on_wait = keep
                new_insts.append(inst)
            bb.instructions = new_insts


# shim antenv.axon_hooks (absent in this image) so BASS_TRACE=1 profiling works
def _install_ntff_shim():
    import sys, types
    if 'antenv.axon_hooks' in sys.modules:
        return
    try:
        mod = types.ModuleType('antenv.axon_hooks')
        state = {}
        mod.set_axon_ntff_profile_hook = lambda h: state.__setitem__('h', h)
        mod.get_axon_ntff_profile_hook = lambda: state.get('h')
        sys.modules['antenv.axon_hooks'] = mod
        import antenv
        antenv.axon_hooks = mod
        from trn_agent_boot.trn_boot import _ntff_profile_via_ctypes
        h = _ntff_profile_via_ctypes('/opt/axon/libaxon_pjrt.so')
        if h is not None:
            mod.set_axon_ntff_profile_hook(h)
    except Exception:
        pass


# ---------------------------------------------------------------------------
def _build_program():
    nc = bass.Bass("TRN2")

    ALU = mybir.AluOpType
    ACT = mybir.ActivationFunctionType

    x1t = nc.dram_tensor("x1t", [D, NK], F16, kind="ExternalInput")
    x2t = nc.dram_tensor("x2t", [D, QSH], F16, kind="ExternalInput")
    # r blocks with an appended ones column: value matmul row 64 yields the
    # softmax denominator (no separate row-sum / sout DMA needed)
    rv8 = nc.dram_tensor("rv8", [128, 8 * 65], BF16, kind="ExternalInput")
    # packed f16 weights: wq1 | wq2 | wk1 | wk2d | ones  -> [64, 321]
    wpack = nc.dram_tensor("wpack", [D, 321], F16, kind="ExternalInput")
    # packed f32 biases: col0 = [bq1; bk1], col1 = bq2d, col2 = bk2d,
    # col3 = -bq2d (for the negated q2t evacuation)
    bpack = nc.dram_tensor("bpack", [128, 4], F32, kind="ExternalInput")
    # lhsT stripe blocks: 16 variants of [128, 32]: block m writes psum rows
    # 2m (partitions 0:64) / 2m+1 (partitions 64:128) of a [32, *] region
    # (base partition must be 0/32/64), coefficient +1.
    labs = nc.dram_tensor("labs", [128, 512], F16, kind="ExternalInput")
    ident = nc.dram_tensor("ident", [D, D], BF16, kind="ExternalInput")
    yout = nc.dram_tensor("yout", [2, 65, 128], F32, kind="ExternalOutput")

    with TileContext(nc) as tc:
        import contextlib
        with contextlib.ExitStack() as ctx:
            consts = ctx.enter_context(tc.tile_pool(name="consts", bufs=1))

            x1t_sb = consts.tile([D, NK], F16)
            x2t_sb = consts.tile([D, QSH], F16)
            r_sb = consts.tile([128, 8 * 65], BF16)
            wpack_sb = consts.tile([D, 321], F16)
            bpack_sb = consts.tile([128, 4], F32)
            labs_sb = consts.tile([128, 512], F16)
            ident_sb = consts.tile([D, D], BF16)

            wq1_sb = wpack_sb[:, 0:64]
            wq2_sb = wpack_sb[:, 64:128]
            wk1_sb = wpack_sb[:, 128:192]
            wk2d_sb = wpack_sb[:, 192:320]
            ones64_sb = wpack_sb[:, 320:321]
            bq1_ap = bpack_sb[0:64, 0:1]
            bk1_ap = bpack_sb[64:128, 0:1]
            bq2d_ap = bpack_sb[:, 1:2]
            bk2d_ap = bpack_sb[:, 2:3]
            bq2dn_ap = bpack_sb[:, 3:4]

            # force the activation-table load to the head of the ACT queue,
            # before any ACT work is otherwise reachable
            dummy_sb = consts.tile([1, 1], F32)
            nc.vector.memset(dummy_sb[:], 0.0)
            nc.scalar.activation(dummy_sb[:], dummy_sb[:], ACT.Relu)

            # warm-up matmuls on zeroed tiles: the PE p-state needs ~3us of
            # continuous execution to reach full clock, so burn the DMA-wait
            # dead time ramping it up
            warm_w = consts.tile([128, 32], F16)
            warm_r = consts.tile([128, 512], F16)
            nc.vector.memset(warm_w[:], 0.0)
            nc.vector.memset(warm_r[:], 0.0)

            # DMA issue order is the schedule: weights and x2t land first in
            # parallel on separate queues so the MLP matmuls start early
            nc.sync.dma_start(out=wpack_sb[:], in_=wpack[:, :])
            nc.sync.dma_start(out=x2t_sb[:], in_=x2t[:, :])
            nc.sync.dma_start(out=x1t_sb[:, 0:512], in_=x1t[:, 0:512])
            nc.sync.dma_start(out=x1t_sb[:, 512:1024], in_=x1t[:, 512:1024])
            nc.scalar.dma_start(out=bpack_sb[:], in_=bpack[:, :])
            nc.scalar.dma_start(out=labs_sb[:], in_=labs[:, :])
            nc.gpsimd.dma_start(out=ident_sb[:], in_=ident[:, :])
            nc.gpsimd.dma_start(out=r_sb[:], in_=rv8[:, :])

            kt2_sb = consts.tile([128, NK], F16)
            q2t_sb = consts.tile([128, 128], F32)
            q2tn_sb = consts.tile([128, 128], F32)
            ht_sb = consts.tile([D, NK], F16)
            hqt_sb = consts.tile([D, QSH], F16)
            expa2_sb = consts.tile([128, 8], F32)
            rsc_sb = consts.tile([128, 8 * 65], BF16)

            # ---- MLPs (transposed), k/q interleaved so the PE fills the
            # ACT-evacuation latency bubbles ----
            with tc.tile_pool(name="mlppsum", bufs=1, space="PSUM") as mp:
                # p-state warm-up on dependency-free zero tiles while the
                # input DMAs land
                wps = mp.tile([32, 512], F32, tag="warm")
                for _ in range(8):
                    nc.tensor.matmul(wps[:], warm_w[:], warm_r[:],
                                     start=True, stop=True,
                                     skip_group_check=True)
                ph0 = mp.tile([D, 512], F32, tag="ph")
                nc.tensor.matmul(ph0[:], wk1_sb, x1t_sb[:, 0:512],
                                 start=True, stop=True)
                phq = mp.tile([D, QSH], F32, tag="phq")
                nc.tensor.matmul(phq[:], wq1_sb, x2t_sb[:], start=True, stop=True)
                nc.scalar.activation(ht_sb[:, 0:512], ph0[:],
                                     ACT.Relu, bias=bk1_ap, scale=1.0)
                nc.scalar.activation(hqt_sb[:], phq[:], ACT.Relu,
                                     bias=bq1_ap, scale=1.0)
                pk0 = mp.tile([128, 512], F32, tag="pk")
                nc.tensor.matmul(pk0[:], wk2d_sb, ht_sb[:, 0:512],
                                 start=True, stop=True)
                pq = mp.tile([128, 128], F32, tag="pq")
                nc.tensor.matmul(pq[0:64, :], wq2_sb, hqt_sb[:, 0:128],
                                 start=True, stop=False, skip_group_check=True)
                nc.tensor.matmul(pq[64:128, :], wq2_sb, hqt_sb[:, 128:256],
                                 start=True, stop=True, skip_group_check=True)
                nc.scalar.activation(kt2_sb[:, 0:512], pk0[:],
                                     ACT.Identity, bias=bk2d_ap, scale=1.0)
                nc.scalar.activation(q2t_sb[:], pq[:], ACT.Identity,
                                     bias=bq2d_ap, scale=1.0)
                nc.scalar.activation(q2tn_sb[:], pq[:], ACT.Identity,
                                     bias=bq2dn_ap, scale=-1.0)
                ph1 = mp.tile([D, 512], F32, tag="ph")
                nc.tensor.matmul(ph1[:], wk1_sb, x1t_sb[:, 512:1024],
                                 start=True, stop=True)
                nc.scalar.activation(ht_sb[:, 512:1024], ph1[:],
                                     ACT.Relu, bias=bk1_ap, scale=1.0)
                pk1 = mp.tile([128, 512], F32, tag="pk")
                nc.tensor.matmul(pk1[:], wk2d_sb, ht_sb[:, 512:1024],
                                 start=True, stop=True)
                nc.scalar.activation(kt2_sb[:, 512:1024], pk1[:],
                                     ACT.Identity, bias=bk2d_ap, scale=1.0)
                # exp(A_j) in key-partition layout, A_j = sum_d k[j, d] from
                # the same f16 kt2 the relu path sees:
                # exp(-dist) = exp(-2 sum_d relu(k-q)) * exp(A_j) * exp(-B_i)
                # (B_i is row-constant and cancels in the normalization);
                # exp(A_j) is folded into the r blocks.
                pa2 = mp.tile([128, 8], F32, tag="pa2")
                for jt in range(8):
                    nc.tensor.matmul(pa2[:, jt:jt + 1],
                                     kt2_sb[0:64, jt * 128:(jt + 1) * 128],
                                     ones64_sb,
                                     start=True, stop=True,
                                     skip_group_check=True)
                nc.scalar.activation(expa2_sb[:], pa2[:], ACT.Exp,
                                     bias=0.0, scale=1.0)
                for jt in range(8):
                    nc.vector.tensor_scalar(
                        rsc_sb[:, jt * 65:(jt + 1) * 65],
                        r_sb[:, jt * 65:(jt + 1) * 65],
                        expa2_sb[:, jt:jt + 1], None, ALU.mult)

            # ---- main loop ----
            mpool = ctx.enter_context(tc.tile_pool(name="mtiles", bufs=8))
            dpool = ctx.enter_context(
                tc.tile_pool(name="dist", bufs=2, space="PSUM"))
            opool = ctx.enter_context(
                tc.tile_pool(name="outp", bufs=2, space="PSUM"))
            vpool = ctx.enter_context(
                tc.tile_pool(name="valp", bufs=2, space="PSUM"))
            spool = ctx.enter_context(tc.tile_pool(name="smax", bufs=2))
            otpool = ctx.enter_context(tc.tile_pool(name="outs", bufs=2))

            def make_tail(rr):
                state = {"expm": [None, None], "expt": None}

                def expf(g, dist):
                    expw = spool.tile([64, NK], BF16, tag=f"expw{g}")
                    state["expm"][g] = expw
                    nc.scalar.activation(expw[:], dist[:], ACT.Exp,
                                         bias=0.0, scale=-2.0)

                def transp(g):
                    if state["expt"] is None:
                        expt = spool.tile([128, 8, 128], BF16, tag="expt")
                        state["expt"] = expt
                    expt = state["expt"]
                    expm = state["expm"][g]
                    tp = opool.tile([128, 8 * D], BF16, tag="outp")
                    for jt in range(8):
                        nc.tensor.transpose(
                            tp[:, jt * D:(jt + 1) * D],
                            expm[:, jt * 128:(jt + 1) * 128],
                            ident_sb[:])
                    nc.vector.tensor_copy(
                        expt[:, :, g * 64:(g + 1) * 64], tp[:])

                def mkvps():
                    vps = vpool.tile([65, 128], F32, tag="vout")
                    state["vps"] = vps

                def value(g):
                    expt = state["expt"]
                    out_ps = state["vps"]
                    for jt in range(8):
                        nc.tensor.matmul(
                            out_ps[:, g * 64:(g + 1) * 64],
                            rsc_sb[:, jt * 65:(jt + 1) * 65],
                            expt[:, jt, g * 64:(g + 1) * 64],
                            start=(jt == 0), stop=(jt == 7),
                            skip_group_check=True)

                def flush():
                    out_ps = state["vps"]
                    ot = otpool.tile([65, 128], F32, tag="ot")
                    nc.scalar.copy(ot[:], out_ps[:])
                    nc.sync.dma_start(out=yout[rr, :, :], in_=ot[:])

                return expf, transp, mkvps, value, flush

            prev = None
            for rr in range(2):
                # the last round runs g=1 first so its exp/transposes/value
                # overlap the g=0 pair matmuls, shortening the final tail
                gorder = (0, 1) if rr == 0 else (1, 0)
                cur = make_tail(rr)
                for pos, g in enumerate(gorder):
                    dist = dpool.tile([64, NK], F32, name="dist", tag="dist")
                    for s in range(32):
                        p = rr * 64 + g * 32 + s
                        base, m = 32 * (s // 16), s % 16
                        mt = mpool.tile([128, NK], F16, tag="mt")
                        if _is_act_pair(p):
                            nc.scalar.activation(mt[:], kt2_sb[:], ACT.Relu,
                                                 bias=q2tn_sb[:, p:p + 1],
                                                 scale=1.0)
                        elif p < 12:
                            # per-window halves: window-0 matmuls start before
                            # the second kt2 window is computed
                            for w in range(NWIN):
                                nc.vector.tensor_scalar(
                                    mt[:, w * 512:(w + 1) * 512],
                                    kt2_sb[:, w * 512:(w + 1) * 512],
                                    q2t_sb[:, p:p + 1], 0.0,
                                    ALU.subtract, ALU.max)
                        else:
                            nc.vector.tensor_scalar(
                                mt[:], kt2_sb[:], q2t_sb[:, p:p + 1], 0.0,
                                ALU.subtract, ALU.max)
                        for w in range(NWIN):
                            nc.tensor.matmul(
                                dist[base:base + 32, w * 512:(w + 1) * 512],
                                labs_sb[:, 32 * m:32 * (m + 1)],
                                mt[:, w * 512:(w + 1) * 512],
                                start=(m == 0), stop=(m == 15),
                                skip_group_check=True)
                        if prev is not None and pos == 0:
                            if s == 4:
                                prev[1](0)     # prev-round transposes
                                prev[1](1)
                            elif s == 10:
                                prev[2]()      # prev-round value psum
                                prev[3](0)
                            elif s == 16:
                                prev[3](1)
                                prev[4]()      # prev-round out copy + DMA
                                prev = None
                        elif rr == 1 and pos == 1:
                            if s == 4:
                                cur[1](1)      # early transposes of g=1
                            elif s == 24:
                                cur[2]()
                                cur[3](1)      # early value matmuls of g=1
                    cur[0](g, dist)            # exp of this group
                prev = cur
            prev[1](0)
            prev[3](0)
            prev[4]()

    _split_excess_waits(nc)
    return nc


_NC_CACHE = None


def _get_nc():
    global _NC_CACHE
    if _NC_CACHE is None:
        _NC_CACHE = _build_program()
    return _NC_CACHE


def kernel(x1, x2, r, Wk1, bk1, Wk2, bk2, Wq1, bq1, Wq2, bq2):
    global LAST_RESULT
    x1 = np.asarray(x1, np.float32)
    x2 = np.asarray(x2, np.float32)
    r = np.asarray(r, np.float32)
    Wk1 = np.asarray(Wk1, np.float32); bk1 = np.asarray(bk1, np.float32)
    Wk2 = np.asarray(Wk2, np.float32); bk2 = np.asarray(bk2, np.float32)
    Wq1 = np.asarray(Wq1, np.float32); bq1 = np.asarray(bq1, np.float32)
    Wq2 = np.asarray(Wq2, np.float32); bq2 = np.asarray(bq2, np.float32)

    # 16 lhsT stripe variants: block m covers cols [32m, 32m+32) with +1 at
    # row 2m (partitions 0:64) / 2m+1 (partitions 64:128)
    labs = np.zeros((128, 512), np.float32)
    for m in range(16):
        labs[0:64, 34 * m] = 1.0
        labs[64:128, 34 * m + 1] = 1.0
    wpack = np.concatenate(
        [Wq1, Wq2, Wk1, np.concatenate([Wk2, Wk2], axis=1),
         np.ones((D, 1), np.float32)], axis=1)
    b2d = np.concatenate([bq2, bq2])
    bpack = np.stack([np.concatenate([bq1, bk1]), b2d,
                      np.concatenate([bk2, bk2]), -b2d], axis=1)
    shared = {
        "wpack": wpack.astype(np.float16),
        "bpack": bpack.astype(np.float32),
        "labs": labs.astype(np.float16),
        "ident": np.eye(D, dtype=ml_dtypes.bfloat16),
    }
    shared = {k: np.ascontiguousarray(v) for k, v in shared.items()}

    in_maps = []
    for c in range(NCORES):
        b, h = c // 2, c % 2
        m = dict(shared)
        m["x1t"] = np.ascontiguousarray(x1[b].T.astype(np.float16))
        m["x2t"] = np.ascontiguousarray(
            x2[b, h * QSH:(h + 1) * QSH].T.astype(np.float16))
        rb = r[b].reshape(8, 128, D).transpose(1, 0, 2)     # [128, 8, 64]
        rb = np.concatenate(
            [rb, np.ones((128, 8, 1), np.float32)], axis=2)  # ones col
        m["rv8"] = np.ascontiguousarray(
            rb.reshape(128, 8 * 65).astype(ml_dtypes.bfloat16))
        in_maps.append(m)

    nc = _get_nc()
    trace = bool(os.environ.get("BASS_TRACE"))
    if trace:
        _install_ntff_shim()
    res = None
    for attempt in range(3):
        try:
            res = bass_utils.run_bass_kernel_spmd(
                nc, in_maps, core_ids=list(range(NCORES)), trace=trace)
            break
        except Exception:
            # transient NRT_EXEC_UNIT_UNRECOVERABLE failures have been
            # observed on this fabric; retry (compile results are cached)
            if attempt == 2:
                raise
            import time
            time.sleep(5)
    LAST_RESULT = res

    # reassemble: yout[rr, :, c]: c = g*64 + u, u = 32b + 2m + i2 -> pair
    # s = 16b + m, local query i2*128 + rr*64 + g*32 + s  (pair p covers
    # queries p and p+128 via the stacked kT2/q2T layout)
    c_idx = np.arange(128)
    g = c_idx // 64
    u = c_idx % 64
    s = 16 * (u // 32) + (u % 32) // 2
    i2 = u % 2
    out = np.empty((B, NQ, D), np.float32)
    for c in range(NCORES):
        b, h = c // 2, c % 2
        yc = res.results[c]["yout"]          # [2, 65, 128]
        for rr in range(2):
            qloc = i2 * 128 + rr * 64 + g * 32 + s
            out[b, h * QSH + qloc, :] = (yc[rr, 0:64] / yc[rr, 64][None, :]).T
    return out


# revision 63
# speedup vs baseline: 1.2460x; 1.0242x over previous
"""Laplace attention kernel for Trainium2, 8 NeuronCores.

Math (per batch b):
  k = MLP_k(x1[b])  [NK, D];  q = MLP_q(x2[b])  [NQ, D]
  dist[i,j] = sum_d |k[j,d] - q[i,d]|
  out = softmax_j(-dist) @ r[b]

Distribution: core c = (b, h) = (c//2, c%2): batch b, query-half h (256 queries).

Per-core algorithm (relu form):
  dist = B_i - A_j + 2*sum_d relu(k_jd - q_id)   (A = sum_d k, B = sum_d q)
  so exp(-dist) = exp(-2*sum relu) * exp(A_j) * exp(-B_i); the exp(-B_i)
  factor is row-constant and cancels in the softmax normalization, and
  exp(A_j) is folded into the value vectors r on device at startup.

  - MLPs run transposed on the PE: kT2 [128=(i2,d), NK] holds kT stacked
    twice, q2T [128=(i2,d), 128] holds qT for query pairs (p, p+128).
  - For each query pair p a [128, NK] tile Mt = relu(kT2 - q_p) is produced
    either on the DVE (chained tensor_scalar (k - q) max 0, 4x f16 mode) or
    on the ACT engine (activation Relu, bias=-q).
  - One PE matmul per 512-column window reduces the 128 partitions to the
    pair's two psum rows out of a 32-row region (psum write base must be
    0/32/64) using one of 16 shared [128, 32] +1-stripe lhsT blocks; 16
    pairs accumulate per region.
  - softmax numerator: ACT Exp (scale=-2) per 32-pair group -> bf16.
  - value: PE transposes of the weights into psum, strided DVE copies into
    a [128, 8, 128] SBUF tile, then accumulating PE matmuls against the
    exp(A)-scaled r blocks, whose appended ones-column yields the softmax
    denominator as output row 64 (no separate row-sum or its DMA).
  - The PE p-state stays at 1.2 GHz until ~25us from kernel start (fixed
    hardware ramp); warm-up matmuls start the PE during the input DMAs and
    the schedule keeps it gap-free so the slow window wastes nothing.
"""

import os
import numpy as np
import ml_dtypes

import concourse.bass as bass
import concourse.mybir as mybir
from concourse.tile import TileContext
from concourse import bass_utils

B, NQ, NK, D = 4, 512, 1024, 64
NCORES = 8
QSH = NQ // 2           # queries per core
NPAIR = QSH // 2        # 128 query pairs per core
NWIN = NK // 512        # 512-column matmul windows

F32 = mybir.dt.float32
F16 = mybir.dt.float16
BF16 = mybir.dt.bfloat16

LAST_RESULT = None      # BassKernelResults of the most recent run (for test.py)

# pairs produced on ACT instead of DVE.  In the first section the ACT
# engine is free right after the MLP evacuations, and the DVE is the
# early-pipeline constraint, so ACT starts earlier there.
ACT_SLOTS = (13, 15, 18, 20, 23, 25, 28, 30)
ACT_SLOTS0 = (5, 7, 9, 11, 13, 15, 18, 20)


def _is_act_pair(p):
    s = p % 32
    return s in (ACT_SLOTS0 if p < 32 else ACT_SLOTS)


# ---------------------------------------------------------------------------
# walrus workaround: the CTRL-class instructions (Drain etc.) can carry only a
# few sem waits; hoist excess waits onto injected NoOps on the same engine.
def _split_excess_waits(nc, max_waits=1):
    for f in nc.m.functions:
        for bb in f.blocks:
            new_insts = []
            for inst in bb.instructions:
                si = inst.sync_info
                if si is not None and si.on_wait and len(si.on_wait) > max_waits:
                    waits = list(si.on_wait)
                    excess, keep = waits[:-max_waits], waits[-max_waits:]
                    for i in range(0, len(excess), max_waits):
                        nop = mybir.InstNoOp(
                            name=f"{inst.name}_waitsplit_{i // max_waits}",
                            ins=[], outs=[])
                        nop.engine = inst.engine
                        nop.sync_info = mybir.SyncInfo(
                            on_wait=excess[i:i + max_waits], on_update=[])
                        new_insts.append(nop)
                    si.on_wait = keep
                new_insts.append(inst)
            bb.instructions = new_insts


# shim antenv.axon_hooks (absent in this image) so BASS_TRACE=1 profiling works
def _install_ntff_shim():
    import sys, types
    if 'antenv.axon_hooks' in sys.modules:
        return
    try:
        mod = types.ModuleType('antenv.axon_hooks')
        state = {}
        mod.set_axon_ntff_profile_hook = lambda h: state.__setitem__('h', h)
        mod.get_axon_ntff_profile_hook = lambda: state.get('h')
        sys.modules['antenv.axon_hooks'] = mod
        import antenv
        antenv.axon_hooks = mod
        from trn_agent_boot.trn_boot import _ntff_profile_via_ctypes
        h = _ntff_profile_via_ctypes('/opt/axon/libaxon_pjrt.so')
        if h is not None:
            mod.set_axon_ntff_profile_hook(h)
    except Exception:
        pass


# ---------------------------------------------------------------------------
def _build_program():
    nc = bass.Bass("TRN2")

    ALU = mybir.AluOpType
    ACT = mybir.ActivationFunctionType

    x1t = nc.dram_tensor("x1t", [D, NK], F16, kind="ExternalInput")
    x2t = nc.dram_tensor("x2t", [D, QSH], F16, kind="ExternalInput")
    # r blocks with an appended ones column: value matmul row 64 yields the
    # softmax denominator (no separate row-sum / sout DMA needed)
    rv8 = nc.dram_tensor("rv8", [128, 8 * 65], BF16, kind="ExternalInput")
    # packed f16 weights: wq1 | wq2 | wk1 | wk2d | ones  -> [64, 321]
    wpack = nc.dram_tensor("wpack", [D, 321], F16, kind="ExternalInput")
    # packed f32 biases: col0 = [bq1; bk1], col1 = bq2d, col2 = bk2d,
    # col3 = -bq2d (for the negated q2t evacuation)
    bpack = nc.dram_tensor("bpack", [128, 4], F32, kind="ExternalInput")
    # lhsT stripe blocks: 16 variants of [128, 32]: block m writes psum rows
    # 2m (partitions 0:64) / 2m+1 (partitions 64:128) of a [32, *] region
    # (base partition must be 0/32/64), coefficient +1.
    labs = nc.dram_tensor("labs", [128, 512], F16, kind="ExternalInput")
    ident = nc.dram_tensor("ident", [D, D], BF16, kind="ExternalInput")
    yout = nc.dram_tensor("yout", [2, 65, 128], F32, kind="ExternalOutput")

    with TileContext(nc) as tc:
        import contextlib
        with contextlib.ExitStack() as ctx:
            consts = ctx.enter_context(tc.tile_pool(name="consts", bufs=1))

            x1t_sb = consts.tile([D, NK], F16)
            x2t_sb = consts.tile([D, QSH], F16)
            r_sb = consts.tile([128, 8 * 65], BF16)
            wpack_sb = consts.tile([D, 321], F16)
            bpack_sb = consts.tile([128, 4], F32)
            labs_sb = consts.tile([128, 512], F16)
            ident_sb = consts.tile([D, D], BF16)

            wq1_sb = wpack_sb[:, 0:64]
            wq2_sb = wpack_sb[:, 64:128]
            wk1_sb = wpack_sb[:, 128:192]
            wk2d_sb = wpack_sb[:, 192:320]
            ones64_sb = wpack_sb[:, 320:321]
            bq1_ap = bpack_sb[0:64, 0:1]
            bk1_ap = bpack_sb[64:128, 0:1]
            bq2d_ap = bpack_sb[:, 1:2]
            bk2d_ap = bpack_sb[:, 2:3]
            bq2dn_ap = bpack_sb[:, 3:4]

            # force the activation-table load to the head of the ACT queue,
            # before any ACT work is otherwise reachable
            dummy_sb = consts.tile([1, 1], F32)
            nc.vector.memset(dummy_sb[:], 0.0)
            nc.scalar.activation(dummy_sb[:], dummy_sb[:], ACT.Relu)

            # warm-up matmuls on zeroed tiles: the PE p-state needs ~3us of
            # continuous execution to reach full clock, so burn the DMA-wait
            # dead time ramping it up
            warm_w = consts.tile([128, 32], F16)
            warm_r = consts.tile([128, 512], F16)
            nc.vector.memset(warm_w[:], 0.0)
            nc.vector.memset(warm_r[:], 0.0)

            # DMA issue order is the schedule: weights and x2t land first in
            # parallel on separate queues so the MLP matmuls start early
            nc.sync.dma_start(out=wpack_sb[:], in_=wpack[:, :])
            nc.sync.dma_start(out=x2t_sb[:], in_=x2t[:, :])
            nc.sync.dma_start(out=x1t_sb[:, 0:512], in_=x1t[:, 0:512])
            nc.sync.dma_start(out=x1t_sb[:, 512:1024], in_=x1t[:, 512:1024])
            nc.scalar.dma_start(out=bpack_sb[:], in_=bpack[:, :])
            nc.scalar.dma_start(out=labs_sb[:], in_=labs[:, :])
            nc.gpsimd.dma_start(out=ident_sb[:], in_=ident[:, :])
            nc.gpsimd.dma_start(out=r_sb[:], in_=rv8[:, :])

            kt2_sb = consts.tile([128, NK], F16)
            q2t_sb = consts.tile([128, 128], F32)
            q2tn_sb = consts.tile([128, 128], F32)
            ht_sb = consts.tile([D, NK], F16)
            hqt_sb = consts.tile([D, QSH], F16)
            expa2_sb = consts.tile([128, 8], F32)
            rsc_sb = consts.tile([128, 8 * 65], BF16)

            # ---- MLPs (transposed), k/q interleaved so the PE fills the
            # ACT-evacuation latency bubbles ----
            with tc.tile_pool(name="mlppsum", bufs=1, space="PSUM") as mp:
                # p-state warm-up on dependency-free zero tiles while the
                # input DMAs land
                wps = mp.tile([32, 512], F32, tag="warm")
                for _ in range(8):
                    nc.tensor.matmul(wps[:], warm_w[:], warm_r[:],
                                     start=True, stop=True,
                                     skip_group_check=True)
                ph0 = mp.tile([D, 512], F32, tag="ph")
                nc.tensor.matmul(ph0[:], wk1_sb, x1t_sb[:, 0:512],
                                 start=True, stop=True)
                phq = mp.tile([D, QSH], F32, tag="phq")
                nc.tensor.matmul(phq[:], wq1_sb, x2t_sb[:], start=True, stop=True)
                nc.scalar.activation(ht_sb[:, 0:512], ph0[:],
                                     ACT.Relu, bias=bk1_ap, scale=1.0)
                nc.scalar.activation(hqt_sb[:], phq[:], ACT.Relu,
                                     bias=bq1_ap, scale=1.0)
                pk0 = mp.tile([128, 512], F32, tag="pk")
                nc.tensor.matmul(pk0[:], wk2d_sb, ht_sb[:, 0:512],
                                 start=True, stop=True)
                pq = mp.tile([128, 128], F32, tag="pq")
                nc.tensor.matmul(pq[0:64, :], wq2_sb, hqt_sb[:, 0:128],
                                 start=True, stop=False, skip_group_check=True)
                nc.tensor.matmul(pq[64:128, :], wq2_sb, hqt_sb[:, 128:256],
                                 start=True, stop=True, skip_group_check=True)
                nc.scalar.activation(kt2_sb[:, 0:512], pk0[:],
                                     ACT.Identity, bias=bk2d_ap, scale=1.0)
                nc.scalar.activation(q2t_sb[:], pq[:], ACT.Identity,
                                     bias=bq2d_ap, scale=1.0)
                nc.scalar.activation(q2tn_sb[:], pq[:], ACT.Identity,
                                     bias=bq2dn_ap, scale=-1.0)
                ph1 = mp.tile([D, 512], F32, tag="ph")
                nc.tensor.matmul(ph1[:], wk1_sb, x1t_sb[:, 512:1024],
                                 start=True, stop=True)
                nc.scalar.activation(ht_sb[:, 512:1024], ph1[:],
                                     ACT.Relu, bias=bk1_ap, scale=1.0)
                pk1 = mp.tile([128, 512], F32, tag="pk")
                nc.tensor.matmul(pk1[:], wk2d_sb, ht_sb[:, 512:1024],
                                 start=True, stop=True)
                nc.scalar.activation(kt2_sb[:, 512:1024], pk1[:],
                                     ACT.Identity, bias=bk2d_ap, scale=1.0)
                # exp(A_j) in key-partition layout, A_j = sum_d k[j, d] from
                # the same f16 kt2 the relu path sees:
                # exp(-dist) = exp(-2 sum_d relu(k-q)) * exp(A_j) * exp(-B_i)
                # (B_i is row-constant and cancels in the normalization);
                # exp(A_j) is folded into the r blocks.
                pa2 = mp.tile([128, 8], F32, tag="pa2")
                for jt in range(8):
                    nc.tensor.matmul(pa2[:, jt:jt + 1],
                                     kt2_sb[0:64, jt * 128:(jt + 1) * 128],
                                     ones64_sb,
                                     start=True, stop=True,
                                     skip_group_check=True)
                nc.scalar.activation(expa2_sb[:], pa2[:], ACT.Exp,
                                     bias=0.0, scale=1.0)
                for jt in range(8):
                    nc.vector.tensor_scalar(
                        rsc_sb[:, jt * 65:(jt + 1) * 65],
                        r_sb[:, jt * 65:(jt + 1) * 65],
                        expa2_sb[:, jt:jt + 1], None, ALU.mult)

            # ---- main loop ----
            mpool = ctx.enter_context(tc.tile_pool(name="mtiles", bufs=8))
            dpool = ctx.enter_context(
                tc.tile_pool(name="dist", bufs=2, space="PSUM"))
            opool = ctx.enter_context(
                tc.tile_pool(name="outp", bufs=2, space="PSUM"))
            vpool = ctx.enter_context(
                tc.tile_pool(name="valp", bufs=2, space="PSUM"))
            spool = ctx.enter_context(tc.tile_pool(name="smax", bufs=2))
            otpool = ctx.enter_context(tc.tile_pool(name="outs", bufs=2))

            def make_tail(rr):
                state = {"expm": [None, None], "expt": None}

                def expf(g, dist):
                    expw = spool.tile([64, NK], BF16, tag=f"expw{g}")
                    state["expm"][g] = expw
                    nc.scalar.activation(expw[:], dist[:], ACT.Exp,
                                         bias=0.0, scale=-2.0)

                def transp(g):
                    if state["expt"] is None:
                        expt = spool.tile([128, 8, 128], BF16, tag="expt")
                        state["expt"] = expt
                    expt = state["expt"]
                    expm = state["expm"][g]
                    tp = opool.tile([128, 8, D], BF16, tag="outp")
                    for jt in range(8):
                        nc.tensor.transpose(
                            tp[:, jt, :],
                            expm[:, jt * 128:(jt + 1) * 128],
                            ident_sb[:])
                    nc.vector.tensor_copy(
                        expt[:, :, g * 64:(g + 1) * 64], tp[:])

                def mkvps():
                    vps = vpool.tile([65, 128], F32, tag="vout")
                    state["vps"] = vps

                def value(g):
                    expt = state["expt"]
                    out_ps = state["vps"]
                    for jt in range(8):
                        nc.tensor.matmul(
                            out_ps[:, g * 64:(g + 1) * 64],
                            rsc_sb[:, jt * 65:(jt + 1) * 65],
                            expt[:, jt, g * 64:(g + 1) * 64],
                            start=(jt == 0), stop=(jt == 7),
                            skip_group_check=True)

                def flush():
                    out_ps = state["vps"]
                    ot = otpool.tile([65, 128], F32, tag="ot")
                    nc.scalar.copy(ot[:], out_ps[:])
                    nc.sync.dma_start(out=yout[rr, :, :], in_=ot[:])

                def expr(g, dist, r):
                    if state["expm"][g] is None:
                        expw = spool.tile([64, NK], BF16, tag=f"expw{g}")
                        state["expm"][g] = expw
                    expw = state["expm"][g]
                    nc.scalar.activation(expw[32 * r:32 * r + 32, :],
                                         dist[32 * r:32 * r + 32, :],
                                         ACT.Exp, bias=0.0, scale=-2.0)

                def transpr(g, r):
                    if state["expt"] is None:
                        expt = spool.tile([128, 8, 128], BF16, tag="expt")
                        state["expt"] = expt
                    expt = state["expt"]
                    expw = state["expm"][g]
                    tpr = opool.tile([128, 8, D], BF16, tag="outp")
                    for jt in range(8):
                        nc.tensor.transpose(
                            tpr[:, jt, 0:32],
                            expw[32 * r:32 * r + 32,
                                 jt * 128:(jt + 1) * 128],
                            ident_sb[32 * r:32 * r + 32, 32 * r:32 * r + 32])
                    c0 = g * 64 + 32 * r
                    nc.vector.tensor_copy(
                        expt[:, :, c0:c0 + 32], tpr[:, :, 0:32])

                def flush_h(h):
                    out_ps = state["vps"]
                    oth = otpool.tile([65, D], F32, tag=f"oth{h}")
                    nc.scalar.copy(oth[:], out_ps[:, h * 64:(h + 1) * 64])
                    nc.sync.dma_start(out=yout[rr, :, h * 64:(h + 1) * 64],
                                      in_=oth[:])

                return expf, transp, mkvps, value, flush, expr, transpr, flush_h

            def emit_producer(p, mt, wins=None):
                if _is_act_pair(p):
                    nc.scalar.activation(mt[:], kt2_sb[:], ACT.Relu,
                                         bias=q2tn_sb[:, p:p + 1], scale=1.0)
                elif wins is None:
                    nc.vector.tensor_scalar(
                        mt[:], kt2_sb[:], q2t_sb[:, p:p + 1], 0.0,
                        ALU.subtract, ALU.max)
                else:
                    for w in wins:
                        nc.vector.tensor_scalar(
                            mt[:, w * 512:(w + 1) * 512],
                            kt2_sb[:, w * 512:(w + 1) * 512],
                            q2t_sb[:, p:p + 1], 0.0, ALU.subtract, ALU.max)

            def emit_matmul(dist, s, mt, w):
                base, m = 32 * (s // 16), s % 16
                nc.tensor.matmul(
                    dist[base:base + 32, w * 512:(w + 1) * 512],
                    labs_sb[:, 32 * m:32 * (m + 1)],
                    mt[:, w * 512:(w + 1) * 512],
                    start=(m == 0), stop=(m == 15), skip_group_check=True)

            PSPLIT = 12
            prev = None
            for rr in range(2):
                # the last round runs g=1 first so its exp/transposes/value
                # overlap the g=0 pair matmuls, shortening the final tail
                gorder = (0, 1) if rr == 0 else (1, 0)
                cur = make_tail(rr)
                for pos, g in enumerate(gorder):
                    dist = dpool.tile([64, NK], F32, name="dist", tag="dist")
                    for s in range(32):
                        p = rr * 64 + g * 32 + s
                        mt = mpool.tile([128, NK], F16, tag="mt")
                        # per-window halves for the earliest pairs: window-0
                        # matmuls start before the second kt2 window exists
                        emit_producer(p, mt, range(NWIN) if p < 12 else None)
                        for w in range(NWIN):
                            emit_matmul(dist, s, mt, w)
                        if prev is not None and pos == 0:
                            if s == 4:
                                prev[1](0)     # prev-round transposes
                                prev[1](1)
                            elif s == 10:
                                prev[2]()      # prev-round value psum
                                prev[3](0)
                            elif s == 16:
                                prev[3](1)
                                prev[4]()      # prev-round out copy + DMA
                                prev = None
                        elif rr == 1 and pos == 1:
                            if s == 4:
                                cur[1](1)      # early transposes of g=1
                            elif s == 24:
                                cur[2]()
                                cur[3](1)      # early value matmuls of g=1
                    cur[0](g, dist)            # exp of this group
                prev = cur
            prev[1](0)
            prev[3](0)
            prev[4]()

    _split_excess_waits(nc)
    return nc


_NC_CACHE = None


def _get_nc():
    global _NC_CACHE
    if _NC_CACHE is None:
        _NC_CACHE = _build_program()
    return _NC_CACHE


def kernel(x1, x2, r, Wk1, bk1, Wk2, bk2, Wq1, bq1, Wq2, bq2):
    global LAST_RESULT
    x1 = np.asarray(x1, np.float32)
    x2 = np.asarray(x2, np.float32)
    r = np.asarray(r, np.float32)
    Wk1 = np.asarray(Wk1, np.float32); bk1 = np.asarray(bk1, np.float32)
    Wk2 = np.asarray(Wk2, np.float32); bk2 = np.asarray(bk2, np.float32)
    Wq1 = np.asarray(Wq1, np.float32); bq1 = np.asarray(bq1, np.float32)
    Wq2 = np.asarray(Wq2, np.float32); bq2 = np.asarray(bq2, np.float32)

    # 16 lhsT stripe variants: block m covers cols [32m, 32m+32) with +1 at
    # row 2m (partitions 0:64) / 2m+1 (partitions 64:128)
    labs = np.zeros((128, 512), np.float32)
    for m in range(16):
        labs[0:64, 34 * m] = 1.0
        labs[64:128, 34 * m + 1] = 1.0
    wpack = np.concatenate(
        [Wq1, Wq2, Wk1, np.concatenate([Wk2, Wk2], axis=1),
         np.ones((D, 1), np.float32)], axis=1)
    b2d = np.concatenate([bq2, bq2])
    bpack = np.stack([np.concatenate([bq1, bk1]), b2d,
                      np.concatenate([bk2, bk2]), -b2d], axis=1)
    shared = {
        "wpack": wpack.astype(np.float16),
        "bpack": bpack.astype(np.float32),
        "labs": labs.astype(np.float16),
        "ident": np.eye(D, dtype=ml_dtypes.bfloat16),
    }
    shared = {k: np.ascontiguousarray(v) for k, v in shared.items()}

    in_maps = []
    for c in range(NCORES):
        b, h = c // 2, c % 2
        m = dict(shared)
        m["x1t"] = np.ascontiguousarray(x1[b].T.astype(np.float16))
        m["x2t"] = np.ascontiguousarray(
            x2[b, h * QSH:(h + 1) * QSH].T.astype(np.float16))
        rb = r[b].reshape(8, 128, D).transpose(1, 0, 2)     # [128, 8, 64]
        rb = np.concatenate(
            [rb, np.ones((128, 8, 1), np.float32)], axis=2)  # ones col
        m["rv8"] = np.ascontiguousarray(
            rb.reshape(128, 8 * 65).astype(ml_dtypes.bfloat16))
        in_maps.append(m)

    nc = _get_nc()
    trace = bool(os.environ.get("BASS_TRACE"))
    if trace:
        _install_ntff_shim()
    res = None
    for attempt in range(3):
        try:
            res = bass_utils.run_bass_kernel_spmd(
                nc, in_maps, core_ids=list(range(NCORES)), trace=trace)
            break
        except Exception:
            # transient NRT_EXEC_UNIT_UNRECOVERABLE failures have been
            # observed on this fabric; retry (compile results are cached)
            if attempt == 2:
                raise
            import time
            time.sleep(5)
    LAST_RESULT = res

    # reassemble: yout[rr, :, c]: c = g*64 + u, u = 32b + 2m + i2 -> pair
    # s = 16b + m, local query i2*128 + rr*64 + g*32 + s  (pair p covers
    # queries p and p+128 via the stacked kT2/q2T layout)
    c_idx = np.arange(128)
    g = c_idx // 64
    u = c_idx % 64
    s = 16 * (u // 32) + (u % 32) // 2
    i2 = u % 2
    out = np.empty((B, NQ, D), np.float32)
    for c in range(NCORES):
        b, h = c // 2, c % 2
        yc = res.results[c]["yout"]          # [2, 65, 128]
        for rr in range(2):
            qloc = i2 * 128 + rr * 64 + g * 32 + s
            out[b, h * QSH + qloc, :] = (yc[rr, 0:64] / yc[rr, 64][None, :]).T
    return out
